# revision 23
# baseline (speedup 1.0000x reference)
"""Trainium2 Bass kernel for nn_Attention_36137854828870 (v3: fp8 attention).

Multi-head causal attention with rotary embeddings, 8 cores:
data-parallel over batch (4) x tensor-parallel over head groups (2x8 heads).
Core c: batch c//2, head group c%2; host sums the two partial outputs.

v3 vs v2: the attention core (QK^T and PV) runs in fp8e4m3 DoubleRow mode
(0.5 cycles/col vs 1.0 for bf16):
 - Q/K are scaled *64 at projection eviction, roped in bf16, written to
   QTb/KTb as fp8.  Each QK^T key tile is ONE DoubleRow matmul whose pair
   slots are stride-0 broadcasts of the same K/Q data (doubles the score;
   folded into the exp scale 2^-16).
 - exp outputs fp8 probabilities directly (unscaled; max p ~37 fits e4m3).
 - V is scaled *64 and stored as an fp8 hi+lo pair, padded to M=128
   stationary tiles [64 V dims | ones col (l row) | 63 zero cols] -- dual
   fp8 weight loads require M in {64,128} fully packed, and cost depends
   only on output free size, so the padding is free.  PV per key-tile-pair
   is two DoubleRow matmuls (hi pass + lo pass).
 - projections and wo stay bf16 (fp8 there fails the absmax budget).
 - normalize is fused into the attnT eviction: reciprocal(l) -> DMA
   broadcast -> tensor_mul(psum, bc) -> attnT bf16 (holds 64*attnout; the
   wo eviction descales by 2^-6).
"""

import os
import sys

sys.path.insert(0, "/opt/trn_rl_repo")

import numpy as np
import ml_dtypes

import concourse.bass as bass
import concourse.mybir as mybir
import concourse.tile as tile
from concourse import bacc
from concourse.bass_utils import run_bass_kernel_spmd

B, S, D = 4, 2048, 1024
H, HD = 16, 64
P = 128
NCORES = 8
HPC = H // 2          # 8 heads per core
DG = HPC * HD         # 512
NKT = D // P          # 8 contraction tiles for projections
NDT = DG // P         # 4 partition-tiles of Q/K/attnT
NSC = 4               # s-chunks == q-blocks
CW = S // NSC         # 512 chunk/q-block width
NST = S // P          # 16 key tiles
F32 = mybir.dt.float32
BF16 = mybir.dt.bfloat16
F8 = mybir.dt.float8e4
DR = mybir.MatmulPerfMode.DoubleRow

SQK = 32.0                    # Q/K/V fp8 pre-scale (applied at eviction)
# NB mybir float8e4 is e4m3 (max finite 240, overflows to inf) -- scale 32
# keeps 32*sqrt(2)*max|q| ~ 157 safely inside.
EXP_SCALE = 1.0 / (2.0 * 8.0 * SQK * SQK)   # bcast-double, sqrt(HD), SQK^2
EVY = 1.0 / SQK               # wo psum -> y descale (attnT = 32*attnout)

_PROGRAM = None
LBL = [""]  # build-time label holder for trace attribution (inert in prod)


def _build_program():
    nc = bacc.Bacc("TRN2", target_bir_lowering=False, debug=False)

    xT_d = nc.dram_tensor("xT", [D, S], BF16, kind="ExternalInput")
    wq_d = nc.dram_tensor("wq", [NDT, P, NKT, P], BF16, kind="ExternalInput")
    wk_d = nc.dram_tensor("wk", [NDT, P, NKT, P], BF16, kind="ExternalInput")
    wv_d = nc.dram_tensor("wv", [D, DG], BF16, kind="ExternalInput")
    wo_d = nc.dram_tensor("wo", [DG, D], BF16, kind="ExternalInput")
    cos_d = nc.dram_tensor("cost", [P, S], BF16, kind="ExternalInput")
    sin_d = nc.dram_tensor("sint", [P, S], BF16, kind="ExternalInput")
    y_d = nc.dram_tensor("y", [S, D], BF16, kind="ExternalOutput")
    DBG = os.environ.get("KV3_DEBUG") == "1"
    if DBG:
        dbg_ktb = nc.dram_tensor("dbg_ktb", [P, NDT, S], F8, kind="ExternalOutput")
        dbg_qtb = nc.dram_tensor("dbg_qtb", [P, NDT, S], F8, kind="ExternalOutput")
        dbg_vh = nc.dram_tensor("dbg_vh", [P, NST // 2, HPC, 2, P], F8,
                                kind="ExternalOutput")
        dbg_vl = nc.dram_tensor("dbg_vl", [P, NST // 2, HPC, 2, P], F8,
                                kind="ExternalOutput")
        dbg_attnT = nc.dram_tensor("dbg_attnT", [P, NDT, S], BF16,
                                   kind="ExternalOutput")
        dbg_ltile = nc.dram_tensor("dbg_ltile", [P, 4, S], BF16,
                                   kind="ExternalOutput")

    xT_v = xT_d.ap().rearrange("(kt p) s -> p kt s", p=P)
    wq_v = wq_d.ap().rearrange("dt p kt m -> p dt kt m")
    wk_v = wk_d.ap().rearrange("dt p kt m -> p dt kt m")
    wv_v = wv_d.ap().rearrange("(kt p) m -> p kt m", p=P)
    wo_v = wo_d.ap().rearrange("(dt p) n -> p dt n", p=P)

    with tile.TileContext(nc) as tc:
        with tc.tile_pool(name="big", bufs=1) as big, \
             tc.tile_pool(name="wres", bufs=1) as wres, \
             tc.tile_pool(name="trig", bufs=1) as trig:
            # persistent SBUF tensors.  V8h/V8l: [p, jp, h, i(pair), m] with
            # m = [64 V dims | ones col | 63 zero pad]; dual-fp8 ldweights
            # needs the (i, m) pair fully packed with m in {64, 128}.
            V8h = big.tile([P, NST // 2, HPC, 2, P], F8, tag="V8h")
            V8l = big.tile([P, NST // 2, HPC, 2, P], F8, tag="V8l")
            KTb = big.tile([P, NDT, S], F8, tag="KTb")
            QTb = big.tile([P, NDT, S], F8, tag="QTb")
            attnT = big.tile([P, NDT, S], BF16, tag="attnT")
            ltile = big.tile([P, 4, S], BF16, tag="ltile")
            ones = big.tile([P, NST * HPC], BF16, tag="ones")
            nc.any.memset(ones[:], 1.0)
            # zero the pad cols + set ones cols (V data cols are fully
            # overwritten by proj_v evictions)
            nc.gpsimd.memset(V8h[:, :, :, :, HD:P], 0.0)
            nc.gpsimd.memset(V8l[:, :, :, :, HD:P], 0.0)
            nc.gpsimd.tensor_copy(
                V8h[:, :, :, :, HD : HD + 1].rearrange(
                    "p a h i o -> p (a h i) o"
                ),
                ones[:].unsqueeze(2),
            )

            wqt = wres.tile([P, NDT, NKT, P], BF16, tag="wq")
            wkt = wres.tile([P, NDT, NKT, P], BF16, tag="wk")
            wvt = wres.tile([P, NKT, DG], BF16, tag="wv")
            wo_sb = wres.tile([P, NDT, D], BF16, tag="wo")
            cost = trig.tile([P, S], BF16, tag="cos")
            sint = trig.tile([P, S], BF16, tag="sin")

            with tc.tile_pool(name="xw", bufs=2) as xw, \
                 tc.tile_pool(name="prj", bufs=2) as prj, \
                 tc.tile_pool(name="prj8", bufs=2) as prj8, \
                 tc.tile_pool(name="scr", bufs=2) as scr, \
                 tc.tile_pool(name="vscr", bufs=2) as vscr, \
                 tc.tile_pool(name="expool", bufs=7) as expool, \
                 tc.tile_pool(name="npool", bufs=3) as npool, \
                 tc.tile_pool(name="ypool", bufs=3) as ypool, \
                 tc.tile_pool(name="ps1", bufs=2, space="PSUM") as ps1, \
                 tc.tile_pool(name="apsum", bufs=2, space="PSUM") as apsum, \
                 tc.tile_pool(name="opsum", bufs=2, space="PSUM") as opsum:

                # first-chunk critical loads: the first proj_dt(dt=0)
                # needs wkt col-slice 0 + xc0 kt-by-kt; split finely and
                # spread across queues, first-used first
                xc0 = xw.tile([P, NKT, CW], BF16, tag="xc")
                nc.scalar.dma_start(out=wkt[:, 0, 0:4, :], in_=wk_v[:, 0, 0:4, :])
                nc.sync.dma_start(out=wkt[:, 0, 4:8, :], in_=wk_v[:, 0, 4:8, :])
                for kt in range(0, NKT, 2):
                    q = (nc.sync, nc.scalar)[(kt // 2) % 2]
                    q.dma_start(
                        out=xc0[:, kt : kt + 2, :], in_=xT_v[:, kt : kt + 2, 0:CW]
                    )
                nc.sync.dma_start(out=cost[:, 0:CW], in_=cos_d.ap()[:, 0:CW])
                nc.scalar.dma_start(out=sint[:, 0:CW], in_=sin_d.ap()[:, 0:CW])
                # half-0 (dt 0,2) weights first: the serial prologue projects
                # K-h0, Q-h0, V before attention(0) starts
                nc.sync.dma_start(out=wkt[:, 2], in_=wk_v[:, 2])
                nc.scalar.dma_start(out=wqt[:, 0], in_=wq_v[:, 0])
                nc.sync.dma_start(out=wqt[:, 2], in_=wq_v[:, 2])
                nc.scalar.dma_start(out=wvt[:, 0:4], in_=wv_v[:, 0:4])
                nc.sync.dma_start(out=wvt[:, 4:8], in_=wv_v[:, 4:8])
                for dt in (1, 3):
                    (nc.sync, nc.scalar)[dt % 2].dma_start(
                        out=wkt[:, dt], in_=wk_v[:, dt]
                    )
                    (nc.scalar, nc.sync)[dt % 2].dma_start(
                        out=wqt[:, dt], in_=wq_v[:, dt]
                    )

                _psq_live = {}

                def proj_dt_half(wt, xc, ta, dt, half):
                    LBL[0] = "projqk"
                    if half == 0:
                        psq = ps1.tile([P, CW], F32, tag="ps")
                        _psq_live[id(ta), dt] = psq
                    psq = _psq_live[id(ta), dt]
                    for kt in range(half * 4, half * 4 + 4):
                        nc.tensor.matmul(
                            psq[:],
                            wt[:, dt, kt, :],
                            xc[:, kt, :],
                            start=(kt == 0),
                            stop=(kt == NKT - 1),
                        )
                    if half == 1:
                        # evict *64: ta holds 64*Qhat ready for fp8
                        nc.vector.tensor_scalar_mul(ta[:, dt, :], psq[:], SQK)
                        del _psq_live[id(ta), dt]

                def rope_half(ta, ta8, csl, half):
                    LBL[0] = "rope"
                    # permA pairing: tiles (half, half+2) hold even/odd
                    # head-dims of the same heads, lane-aligned.  bf16 2x ops
                    # on DVE; the final sub/add write fp8 into ta8 on Pool.
                    a0 = ta[:, half, :]
                    a1 = ta[:, half + 2, :]
                    cc = cost[:, csl]
                    ss = sint[:, csl]
                    tt = scr.tile([P, CW], BF16, tag="t")
                    uu = scr.tile([P, CW], BF16, tag="u")
                    nc.vector.tensor_mul(tt[:], a0, ss)
                    nc.vector.tensor_mul(uu[:], a1, cc)
                    nc.vector.tensor_mul(a0, a0, cc)
                    nc.vector.tensor_mul(a1, a1, ss)
                    nc.gpsimd.tensor_sub(ta8[:, half, :], a0, a1)
                    nc.gpsimd.tensor_add(ta8[:, half + 2, :], tt[:], uu[:])

                def shuffle(ta8, dstb, csl, dtbs):
                    """permA -> permB (head-contiguous) via stride-2-partition
                    SBUF->SBUF DMA on fp8.  permB tile dtb rows: head 2dtb at
                    0-63, head 2dtb+1 at 64-127, even dims on even rows."""
                    LBL[0] = "shuffle"
                    for dtb in dtbs:
                        rlo = 64 * (dtb % 2)
                        for par in range(2):
                            nc.sync.dma_start(
                                out=dstb[par : P : 2, dtb, csl],
                                in_=ta8[rlo : rlo + 64, dtb // 2 + 2 * par, :],
                            )

                def proj_v(xc, c, st):
                    LBL[0] = "projv"
                    psv = ps1.tile([P, DG], F32, tag="ps")
                    for kt in range(NKT):
                        nc.tensor.matmul(
                            psv[:],
                            xc[:, kt, st * P : (st + 1) * P],
                            wvt[:, kt, :],
                            start=(kt == 0),
                            stop=(kt == NKT - 1),
                        )
                    # hi/lo fp8 split of 64*Vhat into the padded V layout
                    j = c * 4 + st
                    vv = vscr.tile([P, DG], BF16, tag="vv")
                    vv3 = vv[:].rearrange("p (h d) -> p h d", h=HPC)
                    vhs = V8h[:, j // 2, :, j % 2, 0:HD]
                    vls = V8l[:, j // 2, :, j % 2, 0:HD]
                    nc.vector.tensor_scalar_mul(vv[:], psv[:], SQK)
                    nc.gpsimd.tensor_copy(vhs, vv3)
                    nc.gpsimd.tensor_sub(vls, vv3, vhs)

                CY = 1.0 / 2.4
                sched = {"pe": 0.0, "act": 0.0}
                pend = []  # exp-done times of in-flight pss tiles (max 2)
                pending = []  # global FIFO of (pe_cost, thunk, tag) filler

                def run_unit(i=0):
                    ucost, u, _tag = pending.pop(i)
                    u()
                    sched["pe"] += ucost * 1.25

                def need(t):
                    while pending and sched["pe"] < t:
                        run_unit()

                def ensure(pred):
                    """Prefix-drain the FIFO through the last unit matching
                    pred -- preserves intra-queue ordering deps."""
                    last = -1
                    for i, (_, _, tag) in enumerate(pending):
                        if pred(tag):
                            last = i
                    for _ in range(last + 1):
                        run_unit()

                def attention(c, dtbs=range(NDT)):
                    """Attention for q-block c.  The global `pending` FIFO
                    holds independent PE work (proj/wo for other blocks); a
                    build-time cursor model of PE and ScalarE completion times
                    pops units exactly where the in-order PE stream would
                    otherwise stall on exp results or PSUM-buffer recycling."""
                    kr = (c + 1) * CW
                    njt = kr // P
                    qb0 = c * CW
                    qsl = slice(qb0, qb0 + CW)

                    for dtb in dtbs:
                        for hh in range(2):
                            pb = hh * 64
                            h = dtb * 2 + hh
                            pso = opsum.tile([P, CW], F32, tag="pso")

                            def emit_pv(pair, first, last, pso=pso, h=h):
                                jp_, qlo0, ex = pair
                                # correctness: chunk jp_//2 V tiles must be
                                # evicted before this PV reads them
                                ensure(lambda tag: tag[0] == "v"
                                       and tag[1] <= jp_ // 2)
                                LBL[0] = f"pv.c{c}"
                                for vt, st_, sp_ in (
                                    (V8h, first, False),
                                    (V8l, False, last),
                                ):
                                    nc.tensor.matmul(
                                        pso[:, qlo0:CW],
                                        vt[:, jp_, h, :, :],
                                        ex[:, :, qlo0:CW],
                                        start=st_,
                                        stop=sp_,
                                        perf_mode=DR,
                                    )

                            pipe = []
                            for jp in range(njt // 2):
                                j0, j1 = 2 * jp, 2 * jp + 1
                                d0 = j0 >= njt - 4
                                d1 = j1 >= njt - 4
                                qlo0 = (j0 - (njt - 4)) * P if d0 else 0
                                qlo1 = (j1 - (njt - 4)) * P if d1 else 0
                                # pss pool bufs=2: this tile recycles the one
                                # whose exp finished 2 pairs ago
                                if len(pend) >= 2:
                                    need(pend.pop(0))
                                pss = apsum.tile([P, 2, CW], F32, tag="pss")
                                LBL[0] = f"qk.c{c}"
                                for i, (jj, qlo) in enumerate(
                                    ((j0, qlo0), (j1, qlo1))
                                ):
                                    nc.tensor.matmul(
                                        pss[:, i, qlo:CW],
                                        KTb[pb : pb + 64, dtb, jj * P : (jj + 1) * P]
                                        .unsqueeze(1)
                                        .broadcast_to((64, 2, P)),
                                        QTb[pb : pb + 64, dtb, qb0 + qlo : qb0 + CW]
                                        .unsqueeze(1)
                                        .broadcast_to((64, 2, CW - qlo)),
                                        start=True,
                                        stop=True,
                                        perf_mode=DR,
                                    )
                                sched["pe"] += (2 * CW - qlo0 - qlo1) * 0.5 * CY
                                ex = expool.tile([P, 2, CW], F8, tag="ex")
                                LBL[0] = f"exp.c{c}"
                                # one exp covers the pair; for a diag pair the
                                # j1 slice [qlo0, qlo1) reads stale PSUM --
                                # bounded garbage, zeroed by the select below
                                nc.scalar.activation(
                                    ex[:, :, qlo0:CW],
                                    pss[:, :, qlo0:CW],
                                    mybir.ActivationFunctionType.Exp,
                                    scale=float(EXP_SCALE),
                                )
                                edone = max(sched["act"], sched["pe"]) + (
                                    2 * (CW - qlo0) + 222
                                ) * 0.833 + 180
                                sched["act"] = edone
                                pend.append(edone)
                                LBL[0] = "sel"
                                if d1:
                                    # one select covers the stale-garbage
                                    # block [qlo0,qlo1) (i-p-128<0 there) and
                                    # the diagonal triangle of j1
                                    w = qlo1 + P - qlo0
                                    nc.gpsimd.affine_select(
                                        out=ex[:, 1, qlo0 : qlo1 + P],
                                        in_=ex[:, 1, qlo0 : qlo1 + P],
                                        compare_op=mybir.AluOpType.is_ge,
                                        fill=0.0,
                                        base=qlo0 - qlo1,
                                        pattern=[[1, w]],
                                        channel_multiplier=-1,
                                    )
                                if d0:
                                    nc.gpsimd.affine_select(
                                        out=ex[:, 0, qlo0 : qlo0 + P],
                                        in_=ex[:, 0, qlo0 : qlo0 + P],
                                        compare_op=mybir.AluOpType.is_ge,
                                        fill=0.0,
                                        base=0,
                                        pattern=[[1, P]],
                                        channel_multiplier=-1,
                                    )
                                eready = edone + (900 if (d0 or d1) else 0)
                                pipe.append((eready, (jp, qlo0, ex), jp))
                                if len(pipe) > 2:
                                    eready, pair, jp_ = pipe.pop(0)
                                    need(eready)
                                    emit_pv(pair, jp_ == 0, False)
                                    sched["pe"] += (CW - pair[1]) * CY
                            np_ = njt // 2
                            for eready, pair, jp_ in pipe:
                                need(eready)
                                emit_pv(pair, jp_ == 0, jp_ == np_ - 1)
                                sched["pe"] += (CW - pair[1]) * CY

                            # fused normalize + eviction: 1/l -> DMA bcast ->
                            # attnT = pso * bc  (attnT holds 64*attnout)
                            LBL[0] = "norm"
                            with nc.allow_low_precision(
                                reason="1/l in bf16; enough for bf16 attnT"
                            ):
                                nc.vector.reciprocal(
                                    ltile[
                                        (h % 2) * 64 : (h % 2) * 64 + 1, h // 2, qsl
                                    ],
                                    pso[HD : HD + 1, :],
                                )
                            bc = npool.tile([P, CW], BF16, tag="bc")
                            nc.gpsimd.dma_start(
                                out=bc[pb : pb + HD, :],
                                in_=ltile[
                                    (h % 2) * 64 : (h % 2) * 64 + 1, h // 2, qsl
                                ]
                                .unsqueeze(1)
                                .broadcast_to((1, HD, CW)),
                            )
                            nc.vector.tensor_mul(
                                attnT[pb : pb + HD, dtb, qsl],
                                pso[0:HD, :],
                                bc[pb : pb + HD, :],
                            )

                _yt_live = {}

                def wo_qt(c, qt, nt, tail=False):
                    LBL[0] = "wo"
                    qlo = c * CW + qt * P
                    if tail and (qt + nt) % 2:
                        psyt = apsum.tile([P, 2, CW], F32, tag="pss")
                        psy = psyt[:, 0, :]
                    else:
                        psyt = ps1.tile([P, CW], F32, tag="ps")
                        psy = psyt[:]
                    for dt in range(NDT):
                        nc.tensor.matmul(
                            psy,
                            attnT[:, dt, qlo : qlo + P],
                            wo_sb[:, dt, nt * CW : (nt + 1) * CW],
                            start=(dt == 0),
                            stop=(dt == NDT - 1),
                        )
                    if nt == 0:
                        yt = ypool.tile([P, 2 * CW], BF16, tag="yt")
                        _yt_live[c, qt] = yt
                    else:
                        yt = _yt_live.pop((c, qt))
                    ysl = yt[:, nt * CW : (nt + 1) * CW]
                    # in the tail phase ScalarE is idle (no more exps); use it
                    # for half the evictions so DVE doesn't serialize the
                    # drain
                    if tail and nt == 1:
                        nc.scalar.mul(ysl, psy, EVY)
                    else:
                        nc.vector.tensor_scalar_mul(ysl, psy, EVY)
                    if nt == 1:
                        nc.sync.dma_start(
                            out=y_d.ap()[qlo : qlo + P, :], in_=yt[:]
                        )

                def proj_units(xc, c):
                    """Tagged thunks projecting chunk c, ordered so the FIFO
                    naturally completes half-0 K/Q and V first: [K-h0, Q-h0,
                    V, K-h1, Q-h1]."""
                    csl = slice(c * CW, (c + 1) * CW)
                    tas = {}
                    for wt, dstb, tn in ((wkt, KTb, "k"), (wqt, QTb, "q")):
                        ta = prj.tile([P, NDT, CW], BF16, tag="ta", name=f"ta_{c}_{tn}")
                        ta8 = prj8.tile([P, NDT, CW], F8, tag="ta8", name=f"ta8_{c}_{tn}")
                        tas[tn] = (ta, ta8, wt, dstb)
                    units = []
                    for half in range(2):
                        for tn in ("k", "q"):
                            ta, ta8, wt, dstb = tas[tn]
                            tag = ("kq", c, half)
                            for dt in (half, half + 2):
                                for kh in range(2):
                                    units.append(
                                        (853, lambda wt=wt, xc=xc, ta=ta,
                                         dt=dt, kh=kh:
                                         proj_dt_half(wt, xc, ta, dt, kh),
                                         tag)
                                    )
                            units.append(
                                (0, lambda ta=ta, ta8=ta8, csl=csl, half=half:
                                 rope_half(ta, ta8, csl, half), tag)
                            )
                            units.append(
                                (0, lambda ta8=ta8, dstb=dstb, csl=csl,
                                 half=half: shuffle(
                                     ta8, dstb, csl,
                                     dtbs=(half * 2, half * 2 + 1)), tag)
                            )
                        if half == 0:
                            for st in range(4):
                                units.append(
                                    (1707, lambda xc=xc, c=c, st=st:
                                     proj_v(xc, c, st), ("v", c))
                                )
                    return units

                def wo_units(c, tail=False):
                    units = []
                    for qt in range(4):
                        for nt in range(2):
                            units.append(
                                (1030, lambda c=c, qt=qt, nt=nt: wo_qt(
                                    c, qt, nt, tail=tail
                                ), ("wo", c))
                            )
                    return units

                def ensure_for(c, halves):
                    def pred(tag):
                        if tag[0] == "kq":
                            return tag[1] < c or (tag[1] == c and
                                                  tag[2] in halves)
                        if tag[0] == "v":
                            return tag[1] <= c
                        return False
                    ensure(pred)

                # Linearized schedule: chunk-0 K-h0/Q-h0/V projected
                # serially, then attention parts stream while the global FIFO
                # drip-feeds proj/wo work into PE stalls.  ensure_for()
                # prefix-drains the FIFO before each part that consumes it.
                xcs = {0: xc0}
                xc1 = xw.tile([P, NKT, CW], BF16, tag="xc")
                xcs[1] = xc1
                pu = proj_units(xc0, 0)
                for ucost, u, _t in pu[0:12]:
                    u()
                    sched["pe"] += ucost
                pending.extend(pu[12:28])
                attention(0, dtbs=range(0, 2))
                nc.sync.dma_start(out=cost[:, CW:S], in_=cos_d.ap()[:, CW:S])
                nc.scalar.dma_start(out=sint[:, CW:S], in_=sin_d.ap()[:, CW:S])
                nc.sync.dma_start(out=xc1[:], in_=xT_v[:, :, CW : 2 * CW])
                nc.scalar.dma_start(out=wo_sb[:], in_=wo_v[:])

                pending.extend(proj_units(xc1, 1))
                xc2 = xw.tile([P, NKT, CW], BF16, tag="xc")
                xcs[2] = xc2
                pending.append(
                    (0, lambda: nc.sync.dma_start(
                        out=xc2[:], in_=xT_v[:, :, 2 * CW : 3 * CW]),
                     ("xc", 2)))
                ensure_for(0, {1})
                attention(0, dtbs=range(2, 4))
                ensure_for(1, {0})
                attention(1, dtbs=range(0, 2))

                pending.extend(proj_units(xc2, 2))
                xc3 = xw.tile([P, NKT, CW], BF16, tag="xc")
                xcs[3] = xc3
                pending.append(
                    (0, lambda: nc.sync.dma_start(
                        out=xc3[:], in_=xT_v[:, :, 3 * CW : 4 * CW]),
                     ("xc", 3)))
                ensure_for(1, {1})
                attention(1, dtbs=range(2, 4))
                ensure_for(2, {0})
                attention(2, dtbs=range(0, 2))

                pending.extend(proj_units(xc3, 3))
                ensure_for(2, {1})
                attention(2, dtbs=range(2, 4))
                pending.extend(wo_units(0) + wo_units(1) + wo_units(2))
                ensure_for(3, {0})
                attention(3, dtbs=range(0, 1))
                ensure_for(3, {0, 1})
                attention(3, dtbs=range(1, 4))
                while pending:
                    run_unit()
                for ucost, u, _t in wo_units(NSC - 1, tail=True):
                    u()
                if os.environ.get("KV3_SCHED") == "1":
                    print(f"[cursor model] pe={sched['pe']:.0f} act={sched['act']:.0f}")
                if DBG:
                    nc.sync.dma_start(out=dbg_ktb.ap(), in_=KTb[:])
                    nc.sync.dma_start(out=dbg_qtb.ap(), in_=QTb[:])
                    nc.sync.dma_start(out=dbg_vh.ap(), in_=V8h[:])
                    nc.sync.dma_start(out=dbg_vl.ap(), in_=V8l[:])
                    nc.sync.dma_start(out=dbg_attnT.ap(), in_=attnT[:])
                    nc.sync.dma_start(out=dbg_ltile.ap(), in_=ltile[:])

    nc.compile()
    return nc


def _perm_a():
    """Column permutation for wq/wk: even head-dims of all heads first
    (head-major, 32 per head), then odd head-dims."""
    perm = np.empty(DG, dtype=np.int64)
    for n in range(DG):
        if n < DG // 2:
            h, i = n // 32, n % 32
            perm[n] = h * HD + 2 * i
        else:
            h, i = (n - DG // 2) // 32, (n - DG // 2) % 32
            perm[n] = h * HD + 2 * i + 1
    return perm


def _pack_w(w):
    """[D, DG] -> [NDT, P(contraction row), NKT, P(col)] so per-dt loads have
    2KB-contiguous runs on both DMA sides."""
    w4 = w.reshape(NKT, P, NDT, P)           # (kt, p, dt, m)
    return np.ascontiguousarray(w4.transpose(2, 1, 0, 3))


def kernel(**inputs):
    global _PROGRAM
    x = np.asarray(inputs["x"], dtype=np.float32)
    freqs_cos = np.asarray(inputs["freqs_cos"], dtype=np.float32)
    freqs_sin = np.asarray(inputs["freqs_sin"], dtype=np.float32)
    wq = np.asarray(inputs["wq"], dtype=np.float32)
    wk = np.asarray(inputs["wk"], dtype=np.float32)
    wv = np.asarray(inputs["wv"], dtype=np.float32)
    wo = np.asarray(inputs["wo"], dtype=np.float32)

    if _PROGRAM is None:
        _PROGRAM = _build_program()
    nc = _PROGRAM

    bf = ml_dtypes.bfloat16
    perm = _perm_a()
    cost = np.ascontiguousarray(np.tile(freqs_cos.T, (4, 1))).astype(bf)
    sint = np.ascontiguousarray(np.tile(freqs_sin.T, (4, 1))).astype(bf)

    in_maps = []
    for c in range(NCORES):
        b, g = c // 2, c % 2
        gsl = slice(g * DG, (g + 1) * DG)
        in_maps.append(
            {
                "xT": np.ascontiguousarray(x[b].T).astype(bf),
                "wq": _pack_w(wq[:, gsl][:, perm]).astype(bf),
                "wk": _pack_w(wk[:, gsl][:, perm]).astype(bf),
                "wv": np.ascontiguousarray(wv[:, gsl]).astype(bf),
                "wo": np.ascontiguousarray(wo[gsl, :]).astype(bf),
                "cost": cost,
                "sint": sint,
            }
        )

    res = run_bass_kernel_spmd(nc, in_maps, list(range(NCORES)))
    global _LAST_RESULTS
    _LAST_RESULTS = res.results
    y = np.empty((B, S, D), dtype=np.float32)
    for b in range(B):
        y[b] = res.results[2 * b]["y"].astype(np.float32) + res.results[
            2 * b + 1
        ]["y"].astype(np.float32)
    return y


# revision 24
# speedup vs baseline: 1.0023x; 1.0023x over previous
"""Trainium2 Bass kernel for nn_Attention_36137854828870 (v3: fp8 attention).

Multi-head causal attention with rotary embeddings, 8 cores:
data-parallel over batch (4) x tensor-parallel over head groups (2x8 heads).
Core c: batch c//2, head group c%2; host sums the two partial outputs.

v3 vs v2: the attention core (QK^T and PV) runs in fp8e4m3 DoubleRow mode
(0.5 cycles/col vs 1.0 for bf16):
 - Q/K are scaled *64 at projection eviction, roped in bf16, written to
   QTb/KTb as fp8.  Each QK^T key tile is ONE DoubleRow matmul whose pair
   slots are stride-0 broadcasts of the same K/Q data (doubles the score;
   folded into the exp scale 2^-16).
 - exp outputs fp8 probabilities directly (unscaled; max p ~37 fits e4m3).
 - V is scaled *64 and stored as an fp8 hi+lo pair, padded to M=128
   stationary tiles [64 V dims | ones col (l row) | 63 zero cols] -- dual
   fp8 weight loads require M in {64,128} fully packed, and cost depends
   only on output free size, so the padding is free.  PV per key-tile-pair
   is two DoubleRow matmuls (hi pass + lo pass).
 - projections and wo stay bf16 (fp8 there fails the absmax budget).
 - normalize is fused into the attnT eviction: reciprocal(l) -> DMA
   broadcast -> tensor_mul(psum, bc) -> attnT bf16 (holds 64*attnout; the
   wo eviction descales by 2^-6).
"""

import os
import sys

sys.path.insert(0, "/opt/trn_rl_repo")

import numpy as np
import ml_dtypes

import concourse.bass as bass
import concourse.mybir as mybir
import concourse.tile as tile
from concourse import bacc
from concourse.bass_utils import run_bass_kernel_spmd

B, S, D = 4, 2048, 1024
H, HD = 16, 64
P = 128
NCORES = 8
HPC = H // 2          # 8 heads per core
DG = HPC * HD         # 512
NKT = D // P          # 8 contraction tiles for projections
NDT = DG // P         # 4 partition-tiles of Q/K/attnT
NSC = 4               # s-chunks == q-blocks
CW = S // NSC         # 512 chunk/q-block width
NST = S // P          # 16 key tiles
F32 = mybir.dt.float32
BF16 = mybir.dt.bfloat16
F8 = mybir.dt.float8e4
DR = mybir.MatmulPerfMode.DoubleRow

SQK = 32.0                    # Q/K/V fp8 pre-scale (applied at eviction)
# NB mybir float8e4 is e4m3 (max finite 240, overflows to inf) -- scale 32
# keeps 32*sqrt(2)*max|q| ~ 157 safely inside.
EXP_SCALE = 1.0 / (2.0 * 8.0 * SQK * SQK)   # bcast-double, sqrt(HD), SQK^2
EVY = 1.0 / SQK               # wo psum -> y descale (attnT = 32*attnout)

_PROGRAM = None
LBL = [""]  # build-time label holder for trace attribution (inert in prod)


def _build_program():
    nc = bacc.Bacc("TRN2", target_bir_lowering=False, debug=False)

    xT_d = nc.dram_tensor("xT", [D, S], BF16, kind="ExternalInput")
    wq_d = nc.dram_tensor("wq", [NDT, P, NKT, P], BF16, kind="ExternalInput")
    wk_d = nc.dram_tensor("wk", [NDT, P, NKT, P], BF16, kind="ExternalInput")
    wv_d = nc.dram_tensor("wv", [D, DG], BF16, kind="ExternalInput")
    wo_d = nc.dram_tensor("wo", [DG, D], BF16, kind="ExternalInput")
    cos_d = nc.dram_tensor("cost", [P, S], BF16, kind="ExternalInput")
    sin_d = nc.dram_tensor("sint", [P, S], BF16, kind="ExternalInput")
    y_d = nc.dram_tensor("y", [S, D], BF16, kind="ExternalOutput")
    DBG = os.environ.get("KV3_DEBUG") == "1"
    if DBG:
        dbg_ktb = nc.dram_tensor("dbg_ktb", [P, NDT, S], F8, kind="ExternalOutput")
        dbg_qtb = nc.dram_tensor("dbg_qtb", [P, NDT, S], F8, kind="ExternalOutput")
        dbg_vh = nc.dram_tensor("dbg_vh", [P, NST // 2, HPC, 2, P], F8,
                                kind="ExternalOutput")
        dbg_vl = nc.dram_tensor("dbg_vl", [P, NST // 2, HPC, 2, P], F8,
                                kind="ExternalOutput")
        dbg_attnT = nc.dram_tensor("dbg_attnT", [P, NDT, S], BF16,
                                   kind="ExternalOutput")
        dbg_ltile = nc.dram_tensor("dbg_ltile", [P, 4, S], BF16,
                                   kind="ExternalOutput")

    xT_v = xT_d.ap().rearrange("(kt p) s -> p kt s", p=P)
    wq_v = wq_d.ap().rearrange("dt p kt m -> p dt kt m")
    wk_v = wk_d.ap().rearrange("dt p kt m -> p dt kt m")
    wv_v = wv_d.ap().rearrange("(kt p) m -> p kt m", p=P)
    wo_v = wo_d.ap().rearrange("(dt p) n -> p dt n", p=P)

    with tile.TileContext(nc) as tc:
        with tc.tile_pool(name="big", bufs=1) as big, \
             tc.tile_pool(name="wres", bufs=1) as wres, \
             tc.tile_pool(name="trig", bufs=1) as trig:
            # persistent SBUF tensors.  V8h/V8l: [p, jp, h, i(pair), m] with
            # m = [64 V dims | ones col | 63 zero pad]; dual-fp8 ldweights
            # needs the (i, m) pair fully packed with m in {64, 128}.
            V8h = big.tile([P, NST // 2, HPC, 2, P], F8, tag="V8h")
            V8l = big.tile([P, NST // 2, HPC, 2, P], F8, tag="V8l")
            KTb = big.tile([P, NDT, S], F8, tag="KTb")
            QTb = big.tile([P, NDT, S], F8, tag="QTb")
            attnT = big.tile([P, NDT, S], BF16, tag="attnT")
            ltile = big.tile([P, 4, S], BF16, tag="ltile")
            ones = big.tile([P, NST * HPC], BF16, tag="ones")
            nc.any.memset(ones[:], 1.0)
            # zero the pad cols + set ones cols (V data cols are fully
            # overwritten by proj_v evictions)
            nc.gpsimd.memset(V8h[:, :, :, :, HD:P], 0.0)
            nc.gpsimd.memset(V8l[:, :, :, :, HD:P], 0.0)
            nc.gpsimd.tensor_copy(
                V8h[:, :, :, :, HD : HD + 1].rearrange(
                    "p a h i o -> p (a h i) o"
                ),
                ones[:].unsqueeze(2),
            )

            wqt = wres.tile([P, NDT, NKT, P], BF16, tag="wq")
            wkt = wres.tile([P, NDT, NKT, P], BF16, tag="wk")
            wvt = wres.tile([P, NKT, DG], BF16, tag="wv")
            wo_sb = wres.tile([P, NDT, D], BF16, tag="wo")
            cost = trig.tile([P, S], BF16, tag="cos")
            sint = trig.tile([P, S], BF16, tag="sin")

            with tc.tile_pool(name="xw", bufs=2) as xw, \
                 tc.tile_pool(name="prj", bufs=2) as prj, \
                 tc.tile_pool(name="prj8", bufs=2) as prj8, \
                 tc.tile_pool(name="scr", bufs=2) as scr, \
                 tc.tile_pool(name="vscr", bufs=2) as vscr, \
                 tc.tile_pool(name="expool", bufs=7) as expool, \
                 tc.tile_pool(name="npool", bufs=3) as npool, \
                 tc.tile_pool(name="ypool", bufs=3) as ypool, \
                 tc.tile_pool(name="ps1", bufs=2, space="PSUM") as ps1, \
                 tc.tile_pool(name="apsum", bufs=2, space="PSUM") as apsum, \
                 tc.tile_pool(name="opsum", bufs=2, space="PSUM") as opsum:

                # first-chunk critical loads: the first proj_dt(dt=0)
                # needs wkt col-slice 0 + xc0 kt-by-kt; split finely and
                # spread across queues, first-used first
                xc0 = xw.tile([P, NKT, CW], BF16, tag="xc")
                nc.scalar.dma_start(out=wkt[:, 0, 0:4, :], in_=wk_v[:, 0, 0:4, :])
                nc.sync.dma_start(out=wkt[:, 0, 4:8, :], in_=wk_v[:, 0, 4:8, :])
                for kt in range(0, NKT, 2):
                    q = (nc.sync, nc.scalar)[(kt // 2) % 2]
                    q.dma_start(
                        out=xc0[:, kt : kt + 2, :], in_=xT_v[:, kt : kt + 2, 0:CW]
                    )
                nc.sync.dma_start(out=cost[:, 0:CW], in_=cos_d.ap()[:, 0:CW])
                nc.scalar.dma_start(out=sint[:, 0:CW], in_=sin_d.ap()[:, 0:CW])
                # half-0 (dt 0,2) weights first: the serial prologue projects
                # K-h0, Q-h0, V before attention(0) starts
                nc.sync.dma_start(out=wkt[:, 2], in_=wk_v[:, 2])
                nc.scalar.dma_start(out=wqt[:, 0], in_=wq_v[:, 0])
                nc.sync.dma_start(out=wqt[:, 2], in_=wq_v[:, 2])
                nc.scalar.dma_start(out=wvt[:, 0:4], in_=wv_v[:, 0:4])
                nc.sync.dma_start(out=wvt[:, 4:8], in_=wv_v[:, 4:8])
                for dt in (1, 3):
                    (nc.sync, nc.scalar)[dt % 2].dma_start(
                        out=wkt[:, dt], in_=wk_v[:, dt]
                    )
                    (nc.scalar, nc.sync)[dt % 2].dma_start(
                        out=wqt[:, dt], in_=wq_v[:, dt]
                    )

                _psq_live = {}

                def proj_dt_half(wt, xc, ta, dt, half):
                    LBL[0] = "projqk"
                    if half == 0:
                        psq = ps1.tile([P, CW], F32, tag="ps")
                        _psq_live[id(ta), dt] = psq
                    psq = _psq_live[id(ta), dt]
                    for kt in range(half * 4, half * 4 + 4):
                        nc.tensor.matmul(
                            psq[:],
                            wt[:, dt, kt, :],
                            xc[:, kt, :],
                            start=(kt == 0),
                            stop=(kt == NKT - 1),
                        )
                    if half == 1:
                        # evict *64: ta holds 64*Qhat ready for fp8
                        nc.vector.tensor_scalar_mul(ta[:, dt, :], psq[:], SQK)
                        del _psq_live[id(ta), dt]

                def rope_half(ta, ta8, csl, half):
                    LBL[0] = "rope"
                    # permA pairing: tiles (half, half+2) hold even/odd
                    # head-dims of the same heads, lane-aligned.  bf16 2x ops
                    # on DVE; the final sub/add write fp8 into ta8 on Pool.
                    a0 = ta[:, half, :]
                    a1 = ta[:, half + 2, :]
                    cc = cost[:, csl]
                    ss = sint[:, csl]
                    tt = scr.tile([P, CW], BF16, tag="t")
                    uu = scr.tile([P, CW], BF16, tag="u")
                    nc.vector.tensor_mul(tt[:], a0, ss)
                    nc.vector.tensor_mul(uu[:], a1, cc)
                    nc.vector.tensor_mul(a0, a0, cc)
                    nc.vector.tensor_mul(a1, a1, ss)
                    nc.gpsimd.tensor_sub(ta8[:, half, :], a0, a1)
                    nc.gpsimd.tensor_add(ta8[:, half + 2, :], tt[:], uu[:])

                def shuffle(ta8, dstb, csl, dtbs):
                    """permA -> permB (head-contiguous) via stride-2-partition
                    SBUF->SBUF DMA on fp8.  permB tile dtb rows: head 2dtb at
                    0-63, head 2dtb+1 at 64-127, even dims on even rows."""
                    LBL[0] = "shuffle"
                    for dtb in dtbs:
                        rlo = 64 * (dtb % 2)
                        for par in range(2):
                            nc.sync.dma_start(
                                out=dstb[par : P : 2, dtb, csl],
                                in_=ta8[rlo : rlo + 64, dtb // 2 + 2 * par, :],
                            )

                def proj_v(xc, c, st):
                    LBL[0] = "projv"
                    psv = ps1.tile([P, DG], F32, tag="ps")
                    for kt in range(NKT):
                        nc.tensor.matmul(
                            psv[:],
                            xc[:, kt, st * P : (st + 1) * P],
                            wvt[:, kt, :],
                            start=(kt == 0),
                            stop=(kt == NKT - 1),
                        )
                    # hi/lo fp8 split of 64*Vhat into the padded V layout
                    j = c * 4 + st
                    vv = vscr.tile([P, DG], BF16, tag="vv")
                    vv3 = vv[:].rearrange("p (h d) -> p h d", h=HPC)
                    vhs = V8h[:, j // 2, :, j % 2, 0:HD]
                    vls = V8l[:, j // 2, :, j % 2, 0:HD]
                    nc.vector.tensor_scalar_mul(vv[:], psv[:], SQK)
                    nc.gpsimd.tensor_copy(vhs, vv3)
                    nc.gpsimd.tensor_sub(vls, vv3, vhs)

                CY = 1.0 / 2.4
                sched = {"pe": 0.0, "act": 0.0}
                pend = []  # exp-done times of in-flight pss tiles (max 2)
                pending = []  # global FIFO of (pe_cost, thunk, tag) filler

                def run_unit(i=0):
                    ucost, u, _tag = pending.pop(i)
                    u()
                    sched["pe"] += ucost * 1.25

                def need(t):
                    while pending and sched["pe"] < t:
                        run_unit()

                def ensure(pred):
                    """Prefix-drain the FIFO through the last unit matching
                    pred -- preserves intra-queue ordering deps."""
                    last = -1
                    for i, (_, _, tag) in enumerate(pending):
                        if pred(tag):
                            last = i
                    for _ in range(last + 1):
                        run_unit()

                def attention(c, dtbs=range(NDT)):
                    """Attention for q-block c.  The global `pending` FIFO
                    holds independent PE work (proj/wo for other blocks); a
                    build-time cursor model of PE and ScalarE completion times
                    pops units exactly where the in-order PE stream would
                    otherwise stall on exp results or PSUM-buffer recycling."""
                    kr = (c + 1) * CW
                    njt = kr // P
                    qb0 = c * CW
                    qsl = slice(qb0, qb0 + CW)

                    for dtb in dtbs:
                        for hh in range(2):
                            pb = hh * 64
                            h = dtb * 2 + hh
                            pso = opsum.tile([P, CW], F32, tag="pso")

                            def emit_pv(pair, first, last, pso=pso, h=h):
                                jp_, qlo0, ex = pair
                                # correctness: chunk jp_//2 V tiles must be
                                # evicted before this PV reads them
                                ensure(lambda tag: tag[0] == "v"
                                       and tag[1] <= jp_ // 2)
                                LBL[0] = f"pv.c{c}"
                                for vt, st_, sp_ in (
                                    (V8h, first, False),
                                    (V8l, False, last),
                                ):
                                    nc.tensor.matmul(
                                        pso[:, qlo0:CW],
                                        vt[:, jp_, h, :, :],
                                        ex[:, :, qlo0:CW],
                                        start=st_,
                                        stop=sp_,
                                        perf_mode=DR,
                                    )

                            pipe = []
                            for jp in range(njt // 2):
                                j0, j1 = 2 * jp, 2 * jp + 1
                                d0 = j0 >= njt - 4
                                d1 = j1 >= njt - 4
                                qlo0 = (j0 - (njt - 4)) * P if d0 else 0
                                qlo1 = (j1 - (njt - 4)) * P if d1 else 0
                                # pss pool bufs=2: this tile recycles the one
                                # whose exp finished 2 pairs ago
                                if len(pend) >= 2:
                                    need(pend.pop(0))
                                pss = apsum.tile([P, 2, CW], F32, tag="pss")
                                LBL[0] = f"qk.c{c}"
                                for i, (jj, qlo) in enumerate(
                                    ((j0, qlo0), (j1, qlo1))
                                ):
                                    nc.tensor.matmul(
                                        pss[:, i, qlo:CW],
                                        KTb[pb : pb + 64, dtb, jj * P : (jj + 1) * P]
                                        .unsqueeze(1)
                                        .broadcast_to((64, 2, P)),
                                        QTb[pb : pb + 64, dtb, qb0 + qlo : qb0 + CW]
                                        .unsqueeze(1)
                                        .broadcast_to((64, 2, CW - qlo)),
                                        start=True,
                                        stop=True,
                                        perf_mode=DR,
                                    )
                                sched["pe"] += (2 * CW - qlo0 - qlo1) * 0.5 * CY
                                ex = expool.tile([P, 2, CW], F8, tag="ex")
                                LBL[0] = f"exp.c{c}"
                                # one exp covers the pair; for a diag pair the
                                # j1 slice [qlo0, qlo1) reads stale PSUM --
                                # bounded garbage, zeroed by the select below
                                nc.scalar.activation(
                                    ex[:, :, qlo0:CW],
                                    pss[:, :, qlo0:CW],
                                    mybir.ActivationFunctionType.Exp,
                                    scale=float(EXP_SCALE),
                                )
                                edone = max(sched["act"], sched["pe"]) + (
                                    2 * (CW - qlo0) + 222
                                ) * 0.833 + 180
                                sched["act"] = edone
                                pend.append(edone)
                                LBL[0] = "sel"
                                if d1:
                                    # one select covers the stale-garbage
                                    # block [qlo0,qlo1) (i-p-128<0 there) and
                                    # the diagonal triangle of j1
                                    w = qlo1 + P - qlo0
                                    nc.gpsimd.affine_select(
                                        out=ex[:, 1, qlo0 : qlo1 + P],
                                        in_=ex[:, 1, qlo0 : qlo1 + P],
                                        compare_op=mybir.AluOpType.is_ge,
                                        fill=0.0,
                                        base=qlo0 - qlo1,
                                        pattern=[[1, w]],
                                        channel_multiplier=-1,
                                    )
                                if d0:
                                    nc.gpsimd.affine_select(
                                        out=ex[:, 0, qlo0 : qlo0 + P],
                                        in_=ex[:, 0, qlo0 : qlo0 + P],
                                        compare_op=mybir.AluOpType.is_ge,
                                        fill=0.0,
                                        base=0,
                                        pattern=[[1, P]],
                                        channel_multiplier=-1,
                                    )
                                eready = edone + (900 if (d0 or d1) else 0)
                                pipe.append((eready, (jp, qlo0, ex), jp))
                                if len(pipe) > 2:
                                    eready, pair, jp_ = pipe.pop(0)
                                    need(eready)
                                    emit_pv(pair, jp_ == 0, False)
                                    sched["pe"] += (CW - pair[1]) * CY
                            np_ = njt // 2
                            for eready, pair, jp_ in pipe:
                                need(eready)
                                emit_pv(pair, jp_ == 0, jp_ == np_ - 1)
                                sched["pe"] += (CW - pair[1]) * CY

                            # fused normalize + eviction: 1/l -> DMA bcast ->
                            # attnT = pso * bc  (attnT holds 64*attnout)
                            LBL[0] = "norm"
                            with nc.allow_low_precision(
                                reason="1/l in bf16; enough for bf16 attnT"
                            ):
                                nc.vector.reciprocal(
                                    ltile[
                                        (h % 2) * 64 : (h % 2) * 64 + 1, h // 2, qsl
                                    ],
                                    pso[HD : HD + 1, :],
                                )
                            bc = npool.tile([P, CW], BF16, tag="bc")
                            nc.gpsimd.dma_start(
                                out=bc[pb : pb + HD, :],
                                in_=ltile[
                                    (h % 2) * 64 : (h % 2) * 64 + 1, h // 2, qsl
                                ]
                                .unsqueeze(1)
                                .broadcast_to((1, HD, CW)),
                            )
                            nc.vector.tensor_mul(
                                attnT[pb : pb + HD, dtb, qsl],
                                pso[0:HD, :],
                                bc[pb : pb + HD, :],
                            )

                _yt_live = {}

                def wo_qt(c, qt, nt, tail=False):
                    LBL[0] = "wo"
                    qlo = c * CW + qt * P
                    if tail and (qt + nt) % 2:
                        psyt = apsum.tile([P, 2, CW], F32, tag="pss")
                        psy = psyt[:, 0, :]
                    else:
                        psyt = ps1.tile([P, CW], F32, tag="ps")
                        psy = psyt[:]
                    for dt in range(NDT):
                        nc.tensor.matmul(
                            psy,
                            attnT[:, dt, qlo : qlo + P],
                            wo_sb[:, dt, nt * CW : (nt + 1) * CW],
                            start=(dt == 0),
                            stop=(dt == NDT - 1),
                        )
                    if nt == 0:
                        yt = ypool.tile([P, 2 * CW], BF16, tag="yt")
                        _yt_live[c, qt] = yt
                    else:
                        yt = _yt_live.pop((c, qt))
                    ysl = yt[:, nt * CW : (nt + 1) * CW]
                    # in the tail phase ScalarE is idle (no more exps); use it
                    # for half the evictions so DVE doesn't serialize the
                    # drain
                    if tail and nt == 1:
                        nc.scalar.mul(ysl, psy, EVY)
                    else:
                        nc.vector.tensor_scalar_mul(ysl, psy, EVY)
                    if nt == 1:
                        nc.sync.dma_start(
                            out=y_d.ap()[qlo : qlo + P, :], in_=yt[:]
                        )

                def proj_units(xc, c):
                    """Tagged thunks projecting chunk c, ordered so the FIFO
                    naturally completes half-0 K/Q and V first: [K-h0, Q-h0,
                    V, K-h1, Q-h1]."""
                    csl = slice(c * CW, (c + 1) * CW)
                    tas = {}
                    for wt, dstb, tn in ((wkt, KTb, "k"), (wqt, QTb, "q")):
                        ta = prj.tile([P, NDT, CW], BF16, tag="ta", name=f"ta_{c}_{tn}")
                        ta8 = prj8.tile([P, NDT, CW], F8, tag="ta8", name=f"ta8_{c}_{tn}")
                        tas[tn] = (ta, ta8, wt, dstb)
                    units = []
                    for half in range(2):
                        for tn in ("k", "q"):
                            ta, ta8, wt, dstb = tas[tn]
                            tag = ("kq", c, half)
                            for dt in (half, half + 2):
                                for kh in range(2):
                                    units.append(
                                        (853, lambda wt=wt, xc=xc, ta=ta,
                                         dt=dt, kh=kh:
                                         proj_dt_half(wt, xc, ta, dt, kh),
                                         tag)
                                    )
                            units.append(
                                (0, lambda ta=ta, ta8=ta8, csl=csl, half=half:
                                 rope_half(ta, ta8, csl, half), tag)
                            )
                            units.append(
                                (0, lambda ta8=ta8, dstb=dstb, csl=csl,
                                 half=half: shuffle(
                                     ta8, dstb, csl,
                                     dtbs=(half * 2, half * 2 + 1)), tag)
                            )
                        if half == 0:
                            for st in range(4):
                                units.append(
                                    (1707, lambda xc=xc, c=c, st=st:
                                     proj_v(xc, c, st), ("v", c))
                                )
                    return units

                def wo_units(c, tail=False):
                    units = []
                    for qt in range(4):
                        for nt in range(2):
                            units.append(
                                (1030, lambda c=c, qt=qt, nt=nt: wo_qt(
                                    c, qt, nt, tail=tail
                                ), ("wo", c))
                            )
                    return units

                def ensure_for(c, halves):
                    def pred(tag):
                        if tag[0] == "kq":
                            return tag[1] < c or (tag[1] == c and
                                                  tag[2] in halves)
                        if tag[0] == "v":
                            return tag[1] <= c
                        return False
                    ensure(pred)

                # Linearized schedule: chunk-0 K-h0/Q-h0/V projected
                # serially, then attention parts stream while the global FIFO
                # drip-feeds proj/wo work into PE stalls.  ensure_for()
                # prefix-drains the FIFO before each part that consumes it.
                xcs = {0: xc0}
                xc1 = xw.tile([P, NKT, CW], BF16, tag="xc")
                xcs[1] = xc1
                pu = proj_units(xc0, 0)
                for ucost, u, _t in pu[0:16]:
                    u()
                    sched["pe"] += ucost
                pending.extend(pu[16:28])
                attention(0, dtbs=range(0, 2))
                nc.sync.dma_start(out=cost[:, CW:S], in_=cos_d.ap()[:, CW:S])
                nc.scalar.dma_start(out=sint[:, CW:S], in_=sin_d.ap()[:, CW:S])
                nc.sync.dma_start(out=xc1[:], in_=xT_v[:, :, CW : 2 * CW])
                nc.scalar.dma_start(out=wo_sb[:], in_=wo_v[:])

                pending.extend(proj_units(xc1, 1))
                xc2 = xw.tile([P, NKT, CW], BF16, tag="xc")
                xcs[2] = xc2
                pending.append(
                    (0, lambda: nc.sync.dma_start(
                        out=xc2[:], in_=xT_v[:, :, 2 * CW : 3 * CW]),
                     ("xc", 2)))
                ensure_for(0, {1})
                attention(0, dtbs=range(2, 4))
                ensure_for(1, {0})
                attention(1, dtbs=range(0, 2))

                pending.extend(proj_units(xc2, 2))
                xc3 = xw.tile([P, NKT, CW], BF16, tag="xc")
                xcs[3] = xc3
                pending.append(
                    (0, lambda: nc.sync.dma_start(
                        out=xc3[:], in_=xT_v[:, :, 3 * CW : 4 * CW]),
                     ("xc", 3)))
                ensure_for(1, {1})
                attention(1, dtbs=range(2, 4))
                ensure_for(2, {0})
                attention(2, dtbs=range(0, 2))

                pending.extend(proj_units(xc3, 3))
                ensure_for(2, {1})
                attention(2, dtbs=range(2, 4))
                pending.extend(wo_units(0) + wo_units(1) + wo_units(2))
                ensure_for(3, {0})
                attention(3, dtbs=range(0, 1))
                ensure_for(3, {0, 1})
                attention(3, dtbs=range(1, 4))
                while pending:
                    run_unit()
                for ucost, u, _t in wo_units(NSC - 1, tail=True):
                    u()
                if os.environ.get("KV3_SCHED") == "1":
                    print(f"[cursor model] pe={sched['pe']:.0f} act={sched['act']:.0f}")
                if DBG:
                    nc.sync.dma_start(out=dbg_ktb.ap(), in_=KTb[:])
                    nc.sync.dma_start(out=dbg_qtb.ap(), in_=QTb[:])
                    nc.sync.dma_start(out=dbg_vh.ap(), in_=V8h[:])
                    nc.sync.dma_start(out=dbg_vl.ap(), in_=V8l[:])
                    nc.sync.dma_start(out=dbg_attnT.ap(), in_=attnT[:])
                    nc.sync.dma_start(out=dbg_ltile.ap(), in_=ltile[:])

    nc.compile()
    return nc


def _perm_a():
    """Column permutation for wq/wk: even head-dims of all heads first
    (head-major, 32 per head), then odd head-dims."""
    perm = np.empty(DG, dtype=np.int64)
    for n in range(DG):
        if n < DG // 2:
            h, i = n // 32, n % 32
            perm[n] = h * HD + 2 * i
        else:
            h, i = (n - DG // 2) // 32, (n - DG // 2) % 32
            perm[n] = h * HD + 2 * i + 1
    return perm


def _pack_w(w):
    """[D, DG] -> [NDT, P(contraction row), NKT, P(col)] so per-dt loads have
    2KB-contiguous runs on both DMA sides."""
    w4 = w.reshape(NKT, P, NDT, P)           # (kt, p, dt, m)
    return np.ascontiguousarray(w4.transpose(2, 1, 0, 3))


def kernel(**inputs):
    global _PROGRAM
    x = np.asarray(inputs["x"], dtype=np.float32)
    freqs_cos = np.asarray(inputs["freqs_cos"], dtype=np.float32)
    freqs_sin = np.asarray(inputs["freqs_sin"], dtype=np.float32)
    wq = np.asarray(inputs["wq"], dtype=np.float32)
    wk = np.asarray(inputs["wk"], dtype=np.float32)
    wv = np.asarray(inputs["wv"], dtype=np.float32)
    wo = np.asarray(inputs["wo"], dtype=np.float32)

    if _PROGRAM is None:
        _PROGRAM = _build_program()
    nc = _PROGRAM

    bf = ml_dtypes.bfloat16
    perm = _perm_a()
    cost = np.ascontiguousarray(np.tile(freqs_cos.T, (4, 1))).astype(bf)
    sint = np.ascontiguousarray(np.tile(freqs_sin.T, (4, 1))).astype(bf)

    in_maps = []
    for c in range(NCORES):
        b, g = c // 2, c % 2
        gsl = slice(g * DG, (g + 1) * DG)
        in_maps.append(
            {
                "xT": np.ascontiguousarray(x[b].T).astype(bf),
                "wq": _pack_w(wq[:, gsl][:, perm]).astype(bf),
                "wk": _pack_w(wk[:, gsl][:, perm]).astype(bf),
                "wv": np.ascontiguousarray(wv[:, gsl]).astype(bf),
                "wo": np.ascontiguousarray(wo[gsl, :]).astype(bf),
                "cost": cost,
                "sint": sint,
            }
        )

    res = run_bass_kernel_spmd(nc, in_maps, list(range(NCORES)))
    global _LAST_RESULTS
    _LAST_RESULTS = res.results
    y = np.empty((B, S, D), dtype=np.float32)
    for b in range(B):
        y[b] = res.results[2 * b]["y"].astype(np.float32) + res.results[
            2 * b + 1
        ]["y"].astype(np.float32)
    return y


# revision 25
# speedup vs baseline: 1.0602x; 1.0577x over previous
"""Trainium2 Bass kernel for nn_Attention_36137854828870 (v3: fp8 attention).

Multi-head causal attention with rotary embeddings, 8 cores:
data-parallel over batch (4) x tensor-parallel over head groups (2x8 heads).
Core c: batch c//2, head group c%2; host sums the two partial outputs.

v3 vs v2: the attention core (QK^T and PV) runs in fp8e4m3 DoubleRow mode
(0.5 cycles/col vs 1.0 for bf16):
 - Q/K are scaled *64 at projection eviction, roped in bf16, written to
   QTb/KTb as fp8.  Each QK^T key tile is ONE DoubleRow matmul whose pair
   slots are stride-0 broadcasts of the same K/Q data (doubles the score;
   folded into the exp scale 2^-16).
 - exp outputs fp8 probabilities directly (unscaled; max p ~37 fits e4m3).
 - V is scaled *64 and stored as an fp8 hi+lo pair, padded to M=128
   stationary tiles [64 V dims | ones col (l row) | 63 zero cols] -- dual
   fp8 weight loads require M in {64,128} fully packed, and cost depends
   only on output free size, so the padding is free.  PV per key-tile-pair
   is two DoubleRow matmuls (hi pass + lo pass).
 - projections and wo stay bf16 (fp8 there fails the absmax budget).
 - normalize is fused into the attnT eviction: reciprocal(l) -> DMA
   broadcast -> tensor_mul(psum, bc) -> attnT bf16 (holds 64*attnout; the
   wo eviction descales by 2^-6).
"""

import os
import sys

sys.path.insert(0, "/opt/trn_rl_repo")

import numpy as np
import ml_dtypes

import concourse.bass as bass
import concourse.mybir as mybir
import concourse.tile as tile
from concourse import bacc
from concourse.bass_utils import run_bass_kernel_spmd

B, S, D = 4, 2048, 1024
H, HD = 16, 64
P = 128
NCORES = 8
HPC = H // 2          # 8 heads per core
DG = HPC * HD         # 512
NKT = D // P          # 8 contraction tiles for projections
NDT = DG // P         # 4 partition-tiles of Q/K/attnT
NSC = 4               # s-chunks == q-blocks
CW = S // NSC         # 512 chunk/q-block width
NST = S // P          # 16 key tiles
F32 = mybir.dt.float32
BF16 = mybir.dt.bfloat16
F8 = mybir.dt.float8e4
DR = mybir.MatmulPerfMode.DoubleRow

SQK = 32.0                    # Q/K/V fp8 pre-scale (applied at eviction)
# NB mybir float8e4 is e4m3 (max finite 240, overflows to inf) -- scale 32
# keeps 32*sqrt(2)*max|q| ~ 157 safely inside.
EXP_SCALE = 1.0 / (2.0 * 8.0 * SQK * SQK)   # bcast-double, sqrt(HD), SQK^2
EVY = 1.0 / SQK               # wo psum -> y descale (attnT = 32*attnout)

_PROGRAM = None
LBL = [""]  # build-time label holder for trace attribution (inert in prod)


def _build_program():
    nc = bacc.Bacc("TRN2", target_bir_lowering=False, debug=False)

    xT_d = nc.dram_tensor("xT", [D, S], BF16, kind="ExternalInput")
    wq_d = nc.dram_tensor("wq", [NDT, P, NKT, P], BF16, kind="ExternalInput")
    wk_d = nc.dram_tensor("wk", [NDT, P, NKT, P], BF16, kind="ExternalInput")
    wv_d = nc.dram_tensor("wv", [D, DG], BF16, kind="ExternalInput")
    wo_d = nc.dram_tensor("wo", [DG, D], BF16, kind="ExternalInput")
    cos_d = nc.dram_tensor("cost", [P, S], BF16, kind="ExternalInput")
    sin_d = nc.dram_tensor("sint", [P, S], BF16, kind="ExternalInput")
    y_d = nc.dram_tensor("y", [S, D], BF16, kind="ExternalOutput")
    DBG = os.environ.get("KV3_DEBUG") == "1"
    if DBG:
        dbg_ktb = nc.dram_tensor("dbg_ktb", [P, NDT, S], F8, kind="ExternalOutput")
        dbg_qtb = nc.dram_tensor("dbg_qtb", [P, NDT, S], F8, kind="ExternalOutput")
        dbg_vh = nc.dram_tensor("dbg_vh", [P, NST // 2, HPC, 2, P], F8,
                                kind="ExternalOutput")
        dbg_vl = nc.dram_tensor("dbg_vl", [P, NST // 2, HPC, 2, P], F8,
                                kind="ExternalOutput")
        dbg_attnT = nc.dram_tensor("dbg_attnT", [P, NDT, S], BF16,
                                   kind="ExternalOutput")
        dbg_ltile = nc.dram_tensor("dbg_ltile", [P, 4, S], BF16,
                                   kind="ExternalOutput")

    xT_v = xT_d.ap().rearrange("(kt p) s -> p kt s", p=P)
    wq_v = wq_d.ap().rearrange("dt p kt m -> p dt kt m")
    wk_v = wk_d.ap().rearrange("dt p kt m -> p dt kt m")
    wv_v = wv_d.ap().rearrange("(kt p) m -> p kt m", p=P)
    wo_v = wo_d.ap().rearrange("(dt p) n -> p dt n", p=P)

    with tile.TileContext(nc) as tc:
        with tc.tile_pool(name="big", bufs=1) as big, \
             tc.tile_pool(name="wres", bufs=1) as wres, \
             tc.tile_pool(name="trig", bufs=1) as trig:
            # persistent SBUF tensors.  V8h/V8l: [p, jp, h, i(pair), m] with
            # m = [64 V dims | ones col | 63 zero pad]; dual-fp8 ldweights
            # needs the (i, m) pair fully packed with m in {64, 128}.
            V8h = big.tile([P, NST // 2, HPC, 2, P], F8, tag="V8h")
            V8l = big.tile([P, NST // 2, HPC, 2, P], F8, tag="V8l")
            KTb = big.tile([P, NDT, S], F8, tag="KTb")
            QTb = big.tile([P, NDT, S], F8, tag="QTb")
            attnT = big.tile([P, NDT, S], BF16, tag="attnT")
            ltile = big.tile([P, 4, S], BF16, tag="ltile")
            ones = big.tile([P, NST * HPC], BF16, tag="ones")
            nc.any.memset(ones[:], 1.0)
            # zero the pad cols + set ones cols (V data cols are fully
            # overwritten by proj_v evictions)
            nc.gpsimd.memset(V8h[:, :, :, :, HD:P], 0.0)
            nc.gpsimd.memset(V8l[:, :, :, :, HD:P], 0.0)
            nc.gpsimd.tensor_copy(
                V8h[:, :, :, :, HD : HD + 1].rearrange(
                    "p a h i o -> p (a h i) o"
                ),
                ones[:].unsqueeze(2),
            )

            wqt = wres.tile([P, NDT, NKT, P], BF16, tag="wq")
            wkt = wres.tile([P, NDT, NKT, P], BF16, tag="wk")
            wvt = wres.tile([P, NKT, DG], BF16, tag="wv")
            wo_sb = wres.tile([P, NDT, D], BF16, tag="wo")
            cost = trig.tile([P, S], BF16, tag="cos")
            sint = trig.tile([P, S], BF16, tag="sin")

            with tc.tile_pool(name="xw", bufs=2) as xw, \
                 tc.tile_pool(name="prj", bufs=2) as prj, \
                 tc.tile_pool(name="prj8", bufs=2) as prj8, \
                 tc.tile_pool(name="scr", bufs=2) as scr, \
                 tc.tile_pool(name="vscr", bufs=2) as vscr, \
                 tc.tile_pool(name="expool", bufs=7) as expool, \
                 tc.tile_pool(name="npool", bufs=3) as npool, \
                 tc.tile_pool(name="ypool", bufs=3) as ypool, \
                 tc.tile_pool(name="ps1", bufs=2, space="PSUM") as ps1, \
                 tc.tile_pool(name="apsum", bufs=2, space="PSUM") as apsum, \
                 tc.tile_pool(name="opsum", bufs=2, space="PSUM") as opsum:

                # first-chunk critical loads: the first proj_dt(dt=0)
                # needs wkt col-slice 0 + xc0 kt-by-kt; split finely and
                # spread across queues, first-used first
                xc0 = xw.tile([P, NKT, CW], BF16, tag="xc")
                nc.scalar.dma_start(out=wkt[:, 0, 0:4, :], in_=wk_v[:, 0, 0:4, :])
                nc.sync.dma_start(out=wkt[:, 0, 4:8, :], in_=wk_v[:, 0, 4:8, :])
                for kt in range(0, NKT, 2):
                    q = (nc.sync, nc.scalar)[(kt // 2) % 2]
                    q.dma_start(
                        out=xc0[:, kt : kt + 2, :], in_=xT_v[:, kt : kt + 2, 0:CW]
                    )
                nc.sync.dma_start(out=cost[:, 0:CW], in_=cos_d.ap()[:, 0:CW])
                nc.scalar.dma_start(out=sint[:, 0:CW], in_=sin_d.ap()[:, 0:CW])
                # half-0 (dt 0,2) weights first: the serial prologue projects
                # K-h0, Q-h0, V before attention(0) starts
                nc.sync.dma_start(out=wkt[:, 2], in_=wk_v[:, 2])
                nc.scalar.dma_start(out=wqt[:, 0], in_=wq_v[:, 0])
                nc.sync.dma_start(out=wqt[:, 2], in_=wq_v[:, 2])
                nc.scalar.dma_start(out=wvt[:, 0:4], in_=wv_v[:, 0:4])
                nc.sync.dma_start(out=wvt[:, 4:8], in_=wv_v[:, 4:8])
                for dt in (1, 3):
                    (nc.sync, nc.scalar)[dt % 2].dma_start(
                        out=wkt[:, dt], in_=wk_v[:, dt]
                    )
                    (nc.scalar, nc.sync)[dt % 2].dma_start(
                        out=wqt[:, dt], in_=wq_v[:, dt]
                    )

                _psq_live = {}

                def proj_dt_half(wt, xc, ta, dt, half):
                    LBL[0] = "projqk"
                    if half == 0:
                        psq = ps1.tile([P, CW], F32, tag="ps")
                        _psq_live[id(ta), dt] = psq
                    psq = _psq_live[id(ta), dt]
                    for kt in range(half * 4, half * 4 + 4):
                        nc.tensor.matmul(
                            psq[:],
                            wt[:, dt, kt, :],
                            xc[:, kt, :],
                            start=(kt == 0),
                            stop=(kt == NKT - 1),
                        )
                    if half == 1:
                        # evict *64: ta holds 64*Qhat ready for fp8
                        nc.vector.tensor_scalar_mul(ta[:, dt, :], psq[:], SQK)
                        del _psq_live[id(ta), dt]

                def rope_half(ta, ta8, csl, half):
                    LBL[0] = "rope"
                    # permA pairing: tiles (half, half+2) hold even/odd
                    # head-dims of the same heads, lane-aligned.  bf16 2x ops
                    # on DVE; the final sub/add write fp8 into ta8 on Pool.
                    a0 = ta[:, half, :]
                    a1 = ta[:, half + 2, :]
                    cc = cost[:, csl]
                    ss = sint[:, csl]
                    tt = scr.tile([P, CW], BF16, tag="t")
                    uu = scr.tile([P, CW], BF16, tag="u")
                    nc.vector.tensor_mul(tt[:], a0, ss)
                    nc.vector.tensor_mul(uu[:], a1, cc)
                    nc.vector.tensor_mul(a0, a0, cc)
                    nc.vector.tensor_mul(a1, a1, ss)
                    nc.gpsimd.tensor_sub(ta8[:, half, :], a0, a1)
                    nc.gpsimd.tensor_add(ta8[:, half + 2, :], tt[:], uu[:])

                def shuffle(ta8, dstb, csl, dtbs):
                    """permA -> permB (head-contiguous) via stride-2-partition
                    SBUF->SBUF DMA on fp8.  permB tile dtb rows: head 2dtb at
                    0-63, head 2dtb+1 at 64-127, even dims on even rows."""
                    LBL[0] = "shuffle"
                    for dtb in dtbs:
                        rlo = 64 * (dtb % 2)
                        for par in range(2):
                            nc.sync.dma_start(
                                out=dstb[par : P : 2, dtb, csl],
                                in_=ta8[rlo : rlo + 64, dtb // 2 + 2 * par, :],
                            )

                def proj_v(xc, c, st):
                    LBL[0] = "projv"
                    psv = ps1.tile([P, DG], F32, tag="ps")
                    for kt in range(NKT):
                        nc.tensor.matmul(
                            psv[:],
                            xc[:, kt, st * P : (st + 1) * P],
                            wvt[:, kt, :],
                            start=(kt == 0),
                            stop=(kt == NKT - 1),
                        )
                    # hi/lo fp8 split of 64*Vhat into the padded V layout
                    j = c * 4 + st
                    vv = vscr.tile([P, DG], BF16, tag="vv")
                    vv3 = vv[:].rearrange("p (h d) -> p h d", h=HPC)
                    vhs = V8h[:, j // 2, :, j % 2, 0:HD]
                    vls = V8l[:, j // 2, :, j % 2, 0:HD]
                    nc.vector.tensor_scalar_mul(vv[:], psv[:], SQK)
                    nc.gpsimd.tensor_copy(vhs, vv3)
                    nc.gpsimd.tensor_sub(vls, vv3, vhs)

                CY = 1.0 / 2.4
                sched = {"pe": 0.0, "act": 0.0}
                pend = []  # exp-done times of in-flight pss tiles (max 2)
                pending = []  # global FIFO of (pe_cost, thunk, tag) filler

                def run_unit(i=0):
                    ucost, u, _tag = pending.pop(i)
                    u()
                    sched["pe"] += ucost * 1.25

                def need(t):
                    while pending and sched["pe"] < t:
                        run_unit()

                def ensure(pred):
                    """Prefix-drain the FIFO through the last unit matching
                    pred -- preserves intra-queue ordering deps."""
                    last = -1
                    for i, (_, _, tag) in enumerate(pending):
                        if pred(tag):
                            last = i
                    for _ in range(last + 1):
                        run_unit()

                def attention(c, dtbs=range(NDT)):
                    """Attention for q-block c.  The global `pending` FIFO
                    holds independent PE work (proj/wo for other blocks); a
                    build-time cursor model of PE and ScalarE completion times
                    pops units exactly where the in-order PE stream would
                    otherwise stall on exp results or PSUM-buffer recycling."""
                    kr = (c + 1) * CW
                    njt = kr // P
                    qb0 = c * CW
                    qsl = slice(qb0, qb0 + CW)

                    for dtb in dtbs:
                        for hh in range(2):
                            pb = hh * 64
                            h = dtb * 2 + hh
                            pso = opsum.tile([P, CW], F32, tag="pso")

                            def emit_pv(pair, first, last, pso=pso, h=h):
                                jp_, qlo0, ex = pair
                                # correctness: chunk jp_//2 V tiles must be
                                # evicted before this PV reads them
                                ensure(lambda tag: tag[0] == "v"
                                       and tag[1] <= jp_ // 2)
                                LBL[0] = f"pv.c{c}"
                                for vt, st_, sp_ in (
                                    (V8h, first, False),
                                    (V8l, False, last),
                                ):
                                    nc.tensor.matmul(
                                        pso[:, qlo0:CW],
                                        vt[:, jp_, h, :, :],
                                        ex[:, :, qlo0:CW],
                                        start=st_,
                                        stop=sp_,
                                        perf_mode=DR,
                                    )

                            pipe = []
                            for jp in range(njt // 2):
                                j0, j1 = 2 * jp, 2 * jp + 1
                                d0 = j0 >= njt - 4
                                d1 = j1 >= njt - 4
                                qlo0 = (j0 - (njt - 4)) * P if d0 else 0
                                qlo1 = (j1 - (njt - 4)) * P if d1 else 0
                                # pss pool bufs=2: this tile recycles the one
                                # whose exp finished 2 pairs ago
                                if len(pend) >= 2:
                                    need(pend.pop(0))
                                pss = apsum.tile([P, 2, CW], F32, tag="pss")
                                LBL[0] = f"qk.c{c}"
                                for i, (jj, qlo) in enumerate(
                                    ((j0, qlo0), (j1, qlo1))
                                ):
                                    nc.tensor.matmul(
                                        pss[:, i, qlo:CW],
                                        KTb[pb : pb + 64, dtb, jj * P : (jj + 1) * P]
                                        .unsqueeze(1)
                                        .broadcast_to((64, 2, P)),
                                        QTb[pb : pb + 64, dtb, qb0 + qlo : qb0 + CW]
                                        .unsqueeze(1)
                                        .broadcast_to((64, 2, CW - qlo)),
                                        start=True,
                                        stop=True,
                                        perf_mode=DR,
                                    )
                                sched["pe"] += (2 * CW - qlo0 - qlo1) * 0.5 * CY
                                ex = expool.tile([P, 2, CW], F8, tag="ex")
                                LBL[0] = f"exp.c{c}"
                                # one exp covers the pair; for a diag pair the
                                # j1 slice [qlo0, qlo1) reads stale PSUM --
                                # bounded garbage, zeroed by the select below
                                nc.scalar.activation(
                                    ex[:, :, qlo0:CW],
                                    pss[:, :, qlo0:CW],
                                    mybir.ActivationFunctionType.Exp,
                                    scale=float(EXP_SCALE),
                                )
                                edone = max(sched["act"], sched["pe"]) + (
                                    2 * (CW - qlo0) + 222
                                ) * 0.833 + 180
                                sched["act"] = edone
                                pend.append(edone)
                                LBL[0] = "sel"
                                if d1:
                                    # one select covers the stale-garbage
                                    # block [qlo0,qlo1) (i-p-128<0 there) and
                                    # the diagonal triangle of j1
                                    w = qlo1 + P - qlo0
                                    nc.gpsimd.affine_select(
                                        out=ex[:, 1, qlo0 : qlo1 + P],
                                        in_=ex[:, 1, qlo0 : qlo1 + P],
                                        compare_op=mybir.AluOpType.is_ge,
                                        fill=0.0,
                                        base=qlo0 - qlo1,
                                        pattern=[[1, w]],
                                        channel_multiplier=-1,
                                    )
                                if d0:
                                    nc.gpsimd.affine_select(
                                        out=ex[:, 0, qlo0 : qlo0 + P],
                                        in_=ex[:, 0, qlo0 : qlo0 + P],
                                        compare_op=mybir.AluOpType.is_ge,
                                        fill=0.0,
                                        base=0,
                                        pattern=[[1, P]],
                                        channel_multiplier=-1,
                                    )
                                eready = edone + (900 if (d0 or d1) else 0)
                                pipe.append((eready, (jp, qlo0, ex), jp))
                                if len(pipe) > 2:
                                    eready, pair, jp_ = pipe.pop(0)
                                    need(eready)
                                    emit_pv(pair, jp_ == 0, False)
                                    sched["pe"] += (CW - pair[1]) * CY
                            np_ = njt // 2
                            for eready, pair, jp_ in pipe:
                                need(eready)
                                emit_pv(pair, jp_ == 0, jp_ == np_ - 1)
                                sched["pe"] += (CW - pair[1]) * CY

                            # fused normalize + eviction: 1/l -> DMA bcast ->
                            # attnT = pso * bc  (attnT holds 64*attnout)
                            LBL[0] = "norm"
                            with nc.allow_low_precision(
                                reason="1/l in bf16; enough for bf16 attnT"
                            ):
                                nc.vector.reciprocal(
                                    ltile[
                                        (h % 2) * 64 : (h % 2) * 64 + 1, h // 2, qsl
                                    ],
                                    pso[HD : HD + 1, :],
                                )
                            bc = npool.tile([P, CW], BF16, tag="bc")
                            nc.sync.dma_start(
                                out=bc[pb : pb + HD, :],
                                in_=ltile[
                                    (h % 2) * 64 : (h % 2) * 64 + 1, h // 2, qsl
                                ]
                                .unsqueeze(1)
                                .broadcast_to((1, HD, CW)),
                            )
                            nc.vector.tensor_mul(
                                attnT[pb : pb + HD, dtb, qsl],
                                pso[0:HD, :],
                                bc[pb : pb + HD, :],
                            )

                _yt_live = {}

                def wo_qt(c, qt, nt, tail=False):
                    LBL[0] = "wo"
                    qlo = c * CW + qt * P
                    if tail and (qt + nt) % 2:
                        psyt = apsum.tile([P, 2, CW], F32, tag="pss")
                        psy = psyt[:, 0, :]
                    else:
                        psyt = ps1.tile([P, CW], F32, tag="ps")
                        psy = psyt[:]
                    for dt in range(NDT):
                        nc.tensor.matmul(
                            psy,
                            attnT[:, dt, qlo : qlo + P],
                            wo_sb[:, dt, nt * CW : (nt + 1) * CW],
                            start=(dt == 0),
                            stop=(dt == NDT - 1),
                        )
                    if nt == 0:
                        yt = ypool.tile([P, 2 * CW], BF16, tag="yt")
                        _yt_live[c, qt] = yt
                    else:
                        yt = _yt_live.pop((c, qt))
                    ysl = yt[:, nt * CW : (nt + 1) * CW]
                    # in the tail phase ScalarE is idle (no more exps); use it
                    # for half the evictions so DVE doesn't serialize the
                    # drain
                    if tail and nt == 1:
                        nc.scalar.mul(ysl, psy, EVY)
                    else:
                        nc.vector.tensor_scalar_mul(ysl, psy, EVY)
                    if nt == 1:
                        nc.sync.dma_start(
                            out=y_d.ap()[qlo : qlo + P, :], in_=yt[:]
                        )

                def proj_units(xc, c):
                    """Tagged thunks projecting chunk c, ordered so the FIFO
                    naturally completes half-0 K/Q and V first: [K-h0, Q-h0,
                    V, K-h1, Q-h1]."""
                    csl = slice(c * CW, (c + 1) * CW)
                    tas = {}
                    for wt, dstb, tn in ((wkt, KTb, "k"), (wqt, QTb, "q")):
                        ta = prj.tile([P, NDT, CW], BF16, tag="ta", name=f"ta_{c}_{tn}")
                        ta8 = prj8.tile([P, NDT, CW], F8, tag="ta8", name=f"ta8_{c}_{tn}")
                        tas[tn] = (ta, ta8, wt, dstb)
                    units = []
                    for half in range(2):
                        for tn in ("k", "q"):
                            ta, ta8, wt, dstb = tas[tn]
                            tag = ("kq", c, half)
                            for dt in (half, half + 2):
                                for kh in range(2):
                                    units.append(
                                        (853, lambda wt=wt, xc=xc, ta=ta,
                                         dt=dt, kh=kh:
                                         proj_dt_half(wt, xc, ta, dt, kh),
                                         tag)
                                    )
                            units.append(
                                (0, lambda ta=ta, ta8=ta8, csl=csl, half=half:
                                 rope_half(ta, ta8, csl, half), tag)
                            )
                            units.append(
                                (0, lambda ta8=ta8, dstb=dstb, csl=csl,
                                 half=half: shuffle(
                                     ta8, dstb, csl,
                                     dtbs=(half * 2, half * 2 + 1)), tag)
                            )
                        if half == 0:
                            for st in range(4):
                                units.append(
                                    (1707, lambda xc=xc, c=c, st=st:
                                     proj_v(xc, c, st), ("v", c))
                                )
                    return units

                def wo_units(c, tail=False):
                    units = []
                    for qt in range(4):
                        for nt in range(2):
                            units.append(
                                (1030, lambda c=c, qt=qt, nt=nt: wo_qt(
                                    c, qt, nt, tail=tail
                                ), ("wo", c))
                            )
                    return units

                def ensure_for(c, halves):
                    def pred(tag):
                        if tag[0] == "kq":
                            return tag[1] < c or (tag[1] == c and
                                                  tag[2] in halves)
                        if tag[0] == "v":
                            return tag[1] <= c
                        return False
                    ensure(pred)

                # Linearized schedule: chunk-0 K-h0/Q-h0/V projected
                # serially, then attention parts stream while the global FIFO
                # drip-feeds proj/wo work into PE stalls.  ensure_for()
                # prefix-drains the FIFO before each part that consumes it.
                xcs = {0: xc0}
                xc1 = xw.tile([P, NKT, CW], BF16, tag="xc")
                xcs[1] = xc1
                pu = proj_units(xc0, 0)
                for ucost, u, _t in pu[0:16]:
                    u()
                    sched["pe"] += ucost
                pending.extend(pu[16:28])
                attention(0, dtbs=range(0, 2))
                nc.sync.dma_start(out=cost[:, CW:S], in_=cos_d.ap()[:, CW:S])
                nc.scalar.dma_start(out=sint[:, CW:S], in_=sin_d.ap()[:, CW:S])
                nc.sync.dma_start(out=xc1[:], in_=xT_v[:, :, CW : 2 * CW])
                nc.scalar.dma_start(out=wo_sb[:], in_=wo_v[:])

                pending.extend(proj_units(xc1, 1))
                xc2 = xw.tile([P, NKT, CW], BF16, tag="xc")
                xcs[2] = xc2
                pending.append(
                    (0, lambda: nc.sync.dma_start(
                        out=xc2[:], in_=xT_v[:, :, 2 * CW : 3 * CW]),
                     ("xc", 2)))
                ensure_for(0, {1})
                attention(0, dtbs=range(2, 4))
                ensure_for(1, {0})
                attention(1, dtbs=range(0, 2))

                pending.extend(proj_units(xc2, 2))
                xc3 = xw.tile([P, NKT, CW], BF16, tag="xc")
                xcs[3] = xc3
                pending.append(
                    (0, lambda: nc.sync.dma_start(
                        out=xc3[:], in_=xT_v[:, :, 3 * CW : 4 * CW]),
                     ("xc", 3)))
                ensure_for(1, {1})
                attention(1, dtbs=range(2, 4))
                ensure_for(2, {0})
                attention(2, dtbs=range(0, 2))

                pending.extend(proj_units(xc3, 3))
                ensure_for(2, {1})
                attention(2, dtbs=range(2, 4))
                pending.extend(wo_units(0) + wo_units(1) + wo_units(2))
                ensure_for(3, {0})
                attention(3, dtbs=range(0, 1))
                ensure_for(3, {0, 1})
                attention(3, dtbs=range(1, 4))
                while pending:
                    run_unit()
                for ucost, u, _t in wo_units(NSC - 1, tail=True):
                    u()
                if os.environ.get("KV3_SCHED") == "1":
                    print(f"[cursor model] pe={sched['pe']:.0f} act={sched['act']:.0f}")
                if DBG:
                    nc.sync.dma_start(out=dbg_ktb.ap(), in_=KTb[:])
                    nc.sync.dma_start(out=dbg_qtb.ap(), in_=QTb[:])
                    nc.sync.dma_start(out=dbg_vh.ap(), in_=V8h[:])
                    nc.sync.dma_start(out=dbg_vl.ap(), in_=V8l[:])
                    nc.sync.dma_start(out=dbg_attnT.ap(), in_=attnT[:])
                    nc.sync.dma_start(out=dbg_ltile.ap(), in_=ltile[:])

    nc.compile()
    return nc


def _perm_a():
    """Column permutation for wq/wk: even head-dims of all heads first
    (head-major, 32 per head), then odd head-dims."""
    perm = np.empty(DG, dtype=np.int64)
    for n in range(DG):
        if n < DG // 2:
            h, i = n // 32, n % 32
            perm[n] = h * HD + 2 * i
        else:
            h, i = (n - DG // 2) // 32, (n - DG // 2) % 32
            perm[n] = h * HD + 2 * i + 1
    return perm


def _pack_w(w):
    """[D, DG] -> [NDT, P(contraction row), NKT, P(col)] so per-dt loads have
    2KB-contiguous runs on both DMA sides."""
    w4 = w.reshape(NKT, P, NDT, P)           # (kt, p, dt, m)
    return np.ascontiguousarray(w4.transpose(2, 1, 0, 3))


def kernel(**inputs):
    global _PROGRAM
    x = np.asarray(inputs["x"], dtype=np.float32)
    freqs_cos = np.asarray(inputs["freqs_cos"], dtype=np.float32)
    freqs_sin = np.asarray(inputs["freqs_sin"], dtype=np.float32)
    wq = np.asarray(inputs["wq"], dtype=np.float32)
    wk = np.asarray(inputs["wk"], dtype=np.float32)
    wv = np.asarray(inputs["wv"], dtype=np.float32)
    wo = np.asarray(inputs["wo"], dtype=np.float32)

    if _PROGRAM is None:
        _PROGRAM = _build_program()
    nc = _PROGRAM

    bf = ml_dtypes.bfloat16
    perm = _perm_a()
    cost = np.ascontiguousarray(np.tile(freqs_cos.T, (4, 1))).astype(bf)
    sint = np.ascontiguousarray(np.tile(freqs_sin.T, (4, 1))).astype(bf)

    in_maps = []
    for c in range(NCORES):
        b, g = c // 2, c % 2
        gsl = slice(g * DG, (g + 1) * DG)
        in_maps.append(
            {
                "xT": np.ascontiguousarray(x[b].T).astype(bf),
                "wq": _pack_w(wq[:, gsl][:, perm]).astype(bf),
                "wk": _pack_w(wk[:, gsl][:, perm]).astype(bf),
                "wv": np.ascontiguousarray(wv[:, gsl]).astype(bf),
                "wo": np.ascontiguousarray(wo[gsl, :]).astype(bf),
                "cost": cost,
                "sint": sint,
            }
        )

    res = run_bass_kernel_spmd(nc, in_maps, list(range(NCORES)))
    global _LAST_RESULTS
    _LAST_RESULTS = res.results
    y = np.empty((B, S, D), dtype=np.float32)
    for b in range(B):
        y[b] = res.results[2 * b]["y"].astype(np.float32) + res.results[
            2 * b + 1
        ]["y"].astype(np.float32)
    return y


# revision 33
# speedup vs baseline: 1.0967x; 1.0344x over previous
"""Trainium2 Bass kernel for nn_Attention_36137854828870 (v3: fp8 attention).

Multi-head causal attention with rotary embeddings, 8 cores:
data-parallel over batch (4) x tensor-parallel over head groups (2x8 heads).
Core c: batch c//2, head group c%2; host sums the two partial outputs.

v3 vs v2: the attention core (QK^T and PV) runs in fp8e4m3 DoubleRow mode
(0.5 cycles/col vs 1.0 for bf16):
 - Q/K are scaled *64 at projection eviction, roped in bf16, written to
   QTb/KTb as fp8.  Each QK^T key tile is ONE DoubleRow matmul whose pair
   slots are stride-0 broadcasts of the same K/Q data (doubles the score;
   folded into the exp scale 2^-16).
 - exp outputs fp8 probabilities directly (unscaled; max p ~37 fits e4m3).
 - V is scaled *64 and stored as an fp8 hi+lo pair, padded to M=128
   stationary tiles [64 V dims | ones col (l row) | 63 zero cols] -- dual
   fp8 weight loads require M in {64,128} fully packed, and cost depends
   only on output free size, so the padding is free.  PV per key-tile-pair
   is two DoubleRow matmuls (hi pass + lo pass).
 - projections and wo stay bf16 (fp8 there fails the absmax budget).
 - normalize is fused into the attnT eviction: reciprocal(l) -> DMA
   broadcast -> tensor_mul(psum, bc) -> attnT bf16 (holds 64*attnout; the
   wo eviction descales by 2^-6).
"""

import os
import sys

sys.path.insert(0, "/opt/trn_rl_repo")

import numpy as np
import ml_dtypes

import concourse.bass as bass
import concourse.mybir as mybir
import concourse.tile as tile
from concourse import bacc
from concourse.bass_utils import run_bass_kernel_spmd

B, S, D = 4, 2048, 1024
H, HD = 16, 64
P = 128
NCORES = 8
HPC = H // 2          # 8 heads per core
DG = HPC * HD         # 512
NKT = D // P          # 8 contraction tiles for projections
NDT = DG // P         # 4 partition-tiles of Q/K/attnT
NSC = 4               # s-chunks == q-blocks
CW = S // NSC         # 512 chunk/q-block width
NST = S // P          # 16 key tiles
F32 = mybir.dt.float32
BF16 = mybir.dt.bfloat16
F8 = mybir.dt.float8e4
DR = mybir.MatmulPerfMode.DoubleRow

SQK = 32.0                    # Q/K/V fp8 pre-scale (applied at eviction)
SX, SW = 32.0, 2048.0         # host pre-scales for x and wq/wk hi/lo splits
EVQK = SQK / (SX * SW)        # fp8-proj psum -> ta descale (ta = 32*Qhat)
# NB mybir float8e4 is e4m3 (max finite 240, overflows to inf) -- scale 32
# keeps 32*sqrt(2)*max|q| ~ 157 safely inside.
EXP_SCALE = 1.0 / (2.0 * 8.0 * SQK * SQK)   # bcast-double, sqrt(HD), SQK^2
EVY = 1.0 / SQK               # wo psum -> y descale (attnT = 32*attnout)

_PROGRAM = None
LBL = [""]  # build-time label holder for trace attribution (inert in prod)
# pacing knobs (env-overridable for calibration sweeps)
INFLATE = float(os.environ.get("KV3_INFLATE", "1.25"))
EXPC = float(os.environ.get("KV3_EXPC", "180"))
PIPED = int(os.environ.get("KV3_PIPED", "2"))


def _build_program():
    nc = bacc.Bacc("TRN2", target_bir_lowering=False, debug=False)

    xT_d = nc.dram_tensor("xT", [D, S], BF16, kind="ExternalInput")
    x8_d = nc.dram_tensor("x8", [NSC, NKT, P, 2, CW], F8, kind="ExternalInput")
    wq8h_d = nc.dram_tensor("wq8h", [NDT, P, NKT, P], F8, kind="ExternalInput")
    wq8l_d = nc.dram_tensor("wq8l", [NDT, P, NKT, P], F8, kind="ExternalInput")
    wk8h_d = nc.dram_tensor("wk8h", [NDT, P, NKT, P], F8, kind="ExternalInput")
    wk8l_d = nc.dram_tensor("wk8l", [NDT, P, NKT, P], F8, kind="ExternalInput")
    wv_d = nc.dram_tensor("wv", [D, DG], BF16, kind="ExternalInput")
    wo_d = nc.dram_tensor("wo", [DG, D], BF16, kind="ExternalInput")
    cos_d = nc.dram_tensor("cost", [P, S], BF16, kind="ExternalInput")
    sin_d = nc.dram_tensor("sint", [P, S], BF16, kind="ExternalInput")
    y_d = nc.dram_tensor("y", [S, D], BF16, kind="ExternalOutput")
    DBG = os.environ.get("KV3_DEBUG") == "1"
    if DBG:
        dbg_ktb = nc.dram_tensor("dbg_ktb", [P, NDT, S], F8, kind="ExternalOutput")
        dbg_qtb = nc.dram_tensor("dbg_qtb", [P, NDT, S], F8, kind="ExternalOutput")
        dbg_vh = nc.dram_tensor("dbg_vh", [P, NST // 2, HPC, 2, P], F8,
                                kind="ExternalOutput")
        dbg_vl = nc.dram_tensor("dbg_vl", [P, NST // 2, HPC, 2, P], F8,
                                kind="ExternalOutput")
        dbg_attnT = nc.dram_tensor("dbg_attnT", [P, NDT, S], BF16,
                                   kind="ExternalOutput")
        dbg_ltile = nc.dram_tensor("dbg_ltile", [P, 4, S], BF16,
                                   kind="ExternalOutput")

    xT_v = xT_d.ap().rearrange("(kt p) s -> p kt s", p=P)
    x8_v = x8_d.ap().rearrange("c kt p t w -> c p kt t w")
    wq8h_v = wq8h_d.ap().rearrange("dt p kt m -> p dt kt m")
    wq8l_v = wq8l_d.ap().rearrange("dt p kt m -> p dt kt m")
    wk8h_v = wk8h_d.ap().rearrange("dt p kt m -> p dt kt m")
    wk8l_v = wk8l_d.ap().rearrange("dt p kt m -> p dt kt m")
    wv_v = wv_d.ap().rearrange("(kt p) m -> p kt m", p=P)
    wo_v = wo_d.ap().rearrange("(dt p) n -> p dt n", p=P)

    with tile.TileContext(nc) as tc:
        with tc.tile_pool(name="big", bufs=1) as big, \
             tc.tile_pool(name="wres", bufs=1) as wres, \
             tc.tile_pool(name="trig", bufs=1) as trig:
            # persistent SBUF tensors.  V8h/V8l: [p, jp, h, i(pair), m] with
            # m = [64 V dims | ones col | 63 zero pad]; dual-fp8 ldweights
            # needs the (i, m) pair fully packed with m in {64, 128}.
            V8h = big.tile([P, NST // 2, HPC, 2, P], F8, tag="V8h")
            V8l = big.tile([P, NST // 2, HPC, 2, P], F8, tag="V8l")
            KTb = big.tile([P, NDT, S], F8, tag="KTb")
            QTb = big.tile([P, NDT, S], F8, tag="QTb")
            attnT = big.tile([P, NDT, S], BF16, tag="attnT")
            ltile = big.tile([P, 4, S], BF16, tag="ltile")
            ones = big.tile([P, NST * HPC], BF16, tag="ones")
            nc.any.memset(ones[:], 1.0)
            # zero the pad cols + set ones cols (V data cols are fully
            # overwritten by proj_v evictions)
            nc.gpsimd.memset(V8h[:, :, :, :, HD:P], 0.0)
            nc.gpsimd.memset(V8l[:, :, :, :, HD:P], 0.0)
            nc.gpsimd.tensor_copy(
                V8h[:, :, :, :, HD : HD + 1].rearrange(
                    "p a h i o -> p (a h i) o"
                ),
                ones[:].unsqueeze(2),
            )

            wqth = wres.tile([P, NDT, NKT, P], F8, tag="wqh")
            wqtl = wres.tile([P, NDT, NKT, P], F8, tag="wql")
            wkth = wres.tile([P, NDT, NKT, P], F8, tag="wkh")
            wktl = wres.tile([P, NDT, NKT, P], F8, tag="wkl")
            wvt = wres.tile([P, NKT, DG], BF16, tag="wv")
            wo_sb = wres.tile([P, NDT, D], BF16, tag="wo")
            cost = trig.tile([P, S], BF16, tag="cos")
            sint = trig.tile([P, S], BF16, tag="sin")

            with tc.tile_pool(name="xw", bufs=2) as xw, \
                 tc.tile_pool(name="xw8", bufs=2) as xw8, \
                 tc.tile_pool(name="prj", bufs=2) as prj, \
                 tc.tile_pool(name="prj8", bufs=2) as prj8, \
                 tc.tile_pool(name="scr", bufs=2) as scr, \
                 tc.tile_pool(name="vscr", bufs=2) as vscr, \
                 tc.tile_pool(name="expool", bufs=7) as expool, \
                 tc.tile_pool(name="npool", bufs=3) as npool, \
                 tc.tile_pool(name="ypool", bufs=3) as ypool, \
                 tc.tile_pool(name="ps1", bufs=2, space="PSUM") as ps1, \
                 tc.tile_pool(name="apsum", bufs=2, space="PSUM") as apsum, \
                 tc.tile_pool(name="opsum", bufs=2, space="PSUM") as opsum:

                # first-chunk critical loads: the first proj_dt(dt=0)
                # needs wkt col-slice 0 + xc0 kt-by-kt; split finely and
                # spread across queues, first-used first
                xc0 = xw.tile([P, NKT, CW], BF16, tag="xc")
                x80 = xw8.tile([P, NKT, 2, CW], F8, tag="x8c")
                nc.scalar.dma_start(out=wkth[:, 0], in_=wk8h_v[:, 0])
                nc.sync.dma_start(out=wktl[:, 0], in_=wk8l_v[:, 0])
                for kt in range(0, NKT, 2):
                    q = (nc.sync, nc.scalar)[(kt // 2) % 2]
                    q.dma_start(
                        out=x80[:, kt : kt + 2], in_=x8_v[0, :, kt : kt + 2]
                    )
                nc.sync.dma_start(out=cost[:, 0:CW], in_=cos_d.ap()[:, 0:CW])
                nc.scalar.dma_start(out=sint[:, 0:CW], in_=sin_d.ap()[:, 0:CW])
                # half-0 (dt 0,2) weights first: the serial prologue projects
                # K-h0, Q-h0 before attention(0) starts
                nc.sync.dma_start(out=wkth[:, 2], in_=wk8h_v[:, 2])
                nc.scalar.dma_start(out=wktl[:, 2], in_=wk8l_v[:, 2])
                nc.sync.dma_start(out=wqth[:, 0], in_=wq8h_v[:, 0])
                nc.scalar.dma_start(out=wqtl[:, 0], in_=wq8l_v[:, 0])
                nc.sync.dma_start(out=wqth[:, 2], in_=wq8h_v[:, 2])
                nc.scalar.dma_start(out=wqtl[:, 2], in_=wq8l_v[:, 2])
                nc.scalar.dma_start(out=wvt[:, 0:4], in_=wv_v[:, 0:4])
                nc.sync.dma_start(out=wvt[:, 4:8], in_=wv_v[:, 4:8])
                for kt in range(0, NKT, 2):
                    q = (nc.scalar, nc.sync)[(kt // 2) % 2]
                    q.dma_start(
                        out=xc0[:, kt : kt + 2, :], in_=xT_v[:, kt : kt + 2, 0:CW]
                    )
                for dt in (1, 3):
                    (nc.sync, nc.scalar)[dt % 2].dma_start(
                        out=wkth[:, dt], in_=wk8h_v[:, dt]
                    )
                    (nc.scalar, nc.sync)[dt % 2].dma_start(
                        out=wktl[:, dt], in_=wk8l_v[:, dt]
                    )
                    (nc.sync, nc.scalar)[dt % 2].dma_start(
                        out=wqth[:, dt], in_=wq8h_v[:, dt]
                    )
                    (nc.scalar, nc.sync)[dt % 2].dma_start(
                        out=wqtl[:, dt], in_=wq8l_v[:, dt]
                    )

                _psq_live = {}

                def proj_dt_half(wth, wtl, x8, ta, dt, half):
                    """3-term fp8 Q/K projection: pass1 (x_hi,x_lo)@W_hi
                    (broadcast dup), pass2 kt-pairs of x_hi @ W_lo pairs.
                    psum = SX*SW*Qhat; evict *EVQK -> ta = 64*Qhat bf16."""
                    LBL[0] = "projqk"
                    if half == 0:
                        psq = ps1.tile([P, CW], F32, tag="ps")
                        _psq_live[id(ta), dt] = psq
                        for kt in range(NKT):
                            nc.tensor.matmul(
                                psq[:],
                                wth[:, dt, kt, :].unsqueeze(1)
                                .broadcast_to((P, 2, P)),
                                x8[:, kt],
                                start=(kt == 0),
                                stop=False,
                                perf_mode=DR,
                            )
                        return
                    psq = _psq_live[id(ta), dt]
                    for ktp in range(NKT // 2):
                        nc.tensor.matmul(
                            psq[:],
                            wtl[:, dt, 2 * ktp : 2 * ktp + 2, :],
                            x8[:, 2 * ktp : 2 * ktp + 2, 0, :],
                            start=False,
                            stop=(ktp == NKT // 2 - 1),
                            perf_mode=DR,
                        )
                    nc.vector.tensor_scalar_mul(ta[:, dt, :], psq[:], EVQK)
                    del _psq_live[id(ta), dt]

                def rope_half(ta, ta8, csl, half):
                    LBL[0] = "rope"
                    # permA pairing: tiles (half, half+2) hold even/odd
                    # head-dims of the same heads, lane-aligned.  bf16 2x ops
                    # on DVE; the final sub/add write fp8 into ta8 on Pool.
                    a0 = ta[:, half, :]
                    a1 = ta[:, half + 2, :]
                    cc = cost[:, csl]
                    ss = sint[:, csl]
                    tt = scr.tile([P, CW], BF16, tag="t")
                    uu = scr.tile([P, CW], BF16, tag="u")
                    nc.vector.tensor_mul(tt[:], a0, ss)
                    nc.vector.tensor_mul(uu[:], a1, cc)
                    nc.vector.tensor_mul(a0, a0, cc)
                    nc.vector.tensor_mul(a1, a1, ss)
                    nc.gpsimd.tensor_sub(ta8[:, half, :], a0, a1)
                    nc.gpsimd.tensor_add(ta8[:, half + 2, :], tt[:], uu[:])

                def shuffle(ta8, dstb, csl, dtbs):
                    """permA -> permB (head-contiguous) via stride-2-partition
                    SBUF->SBUF DMA on fp8.  permB tile dtb rows: head 2dtb at
                    0-63, head 2dtb+1 at 64-127, even dims on even rows."""
                    LBL[0] = "shuffle"
                    for dtb in dtbs:
                        rlo = 64 * (dtb % 2)
                        for par in range(2):
                            nc.sync.dma_start(
                                out=dstb[par : P : 2, dtb, csl],
                                in_=ta8[rlo : rlo + 64, dtb // 2 + 2 * par, :],
                            )

                _psv_live = {}

                def proj_v_half(xc, c, st, half):
                    LBL[0] = "projv"
                    if half == 0:
                        _psv_live[id(xc), st] = ps1.tile([P, DG], F32, tag="ps", name="psv")
                    psv = _psv_live[id(xc), st]
                    for kt in range(half * 4, half * 4 + 4):
                        nc.tensor.matmul(
                            psv[:],
                            xc[:, kt, st * P : (st + 1) * P],
                            wvt[:, kt, :],
                            start=(kt == 0),
                            stop=(kt == NKT - 1),
                        )
                    if half == 0:
                        return
                    del _psv_live[id(xc), st]
                    # hi/lo fp8 split of 64*Vhat into the padded V layout
                    j = c * 4 + st
                    vv = vscr.tile([P, DG], BF16, tag="vv")
                    vv3 = vv[:].rearrange("p (h d) -> p h d", h=HPC)
                    vhs = V8h[:, j // 2, :, j % 2, 0:HD]
                    vls = V8l[:, j // 2, :, j % 2, 0:HD]
                    nc.vector.tensor_scalar_mul(vv[:], psv[:], SQK)
                    nc.gpsimd.tensor_copy(vhs, vv3)
                    nc.gpsimd.tensor_sub(vls, vv3, vhs)

                CY = 1.0 / 2.4
                sched = {"pe": 0.0, "act": 0.0}
                pend = []  # exp-done times of in-flight pss tiles (max 2)
                pending = []  # global FIFO of (pe_cost, thunk, tag) filler

                def run_unit(i=0):
                    ucost, u, _tag = pending.pop(i)
                    u()
                    sched["pe"] += ucost * INFLATE

                def need(t):
                    while pending and sched["pe"] < t:
                        run_unit()

                def ensure(pred):
                    """Prefix-drain the FIFO through the last unit matching
                    pred -- preserves intra-queue ordering deps."""
                    last = -1
                    for i, (_, _, tag) in enumerate(pending):
                        if pred(tag):
                            last = i
                    for _ in range(last + 1):
                        run_unit()

                def attention(c, dtbs=range(NDT)):
                    """Attention for q-block c.  The global `pending` FIFO
                    holds independent PE work (proj/wo for other blocks); a
                    build-time cursor model of PE and ScalarE completion times
                    pops units exactly where the in-order PE stream would
                    otherwise stall on exp results or PSUM-buffer recycling."""
                    kr = (c + 1) * CW
                    njt = kr // P
                    qb0 = c * CW
                    qsl = slice(qb0, qb0 + CW)

                    for dtb in dtbs:
                        for hh in range(2):
                            pb = hh * 64
                            h = dtb * 2 + hh
                            pso = opsum.tile([P, CW], F32, tag="pso")

                            def emit_pv(pair, first, last, pso=pso, h=h):
                                jp_, qlo0, ex = pair
                                # correctness: chunk jp_//2 V tiles must be
                                # evicted before this PV reads them
                                ensure(lambda tag: tag[0] == "v"
                                       and tag[1] <= jp_ // 2)
                                LBL[0] = f"pv.c{c}"
                                for vt, st_, sp_ in (
                                    (V8h, first, False),
                                    (V8l, False, last),
                                ):
                                    nc.tensor.matmul(
                                        pso[:, qlo0:CW],
                                        vt[:, jp_, h, :, :],
                                        ex[:, :, qlo0:CW],
                                        start=st_,
                                        stop=sp_,
                                        perf_mode=DR,
                                    )

                            pipe = []
                            for jp in range(njt // 2):
                                j0, j1 = 2 * jp, 2 * jp + 1
                                d0 = j0 >= njt - 4
                                d1 = j1 >= njt - 4
                                qlo0 = (j0 - (njt - 4)) * P if d0 else 0
                                qlo1 = (j1 - (njt - 4)) * P if d1 else 0
                                # pss pool bufs=2: this tile recycles the one
                                # whose exp finished 2 pairs ago
                                if len(pend) >= 2:
                                    need(pend.pop(0))
                                pss = apsum.tile([P, 2, CW], F32, tag="pss")
                                LBL[0] = f"qk.c{c}"
                                for i, (jj, qlo) in enumerate(
                                    ((j0, qlo0), (j1, qlo1))
                                ):
                                    nc.tensor.matmul(
                                        pss[:, i, qlo:CW],
                                        KTb[pb : pb + 64, dtb, jj * P : (jj + 1) * P]
                                        .unsqueeze(1)
                                        .broadcast_to((64, 2, P)),
                                        QTb[pb : pb + 64, dtb, qb0 + qlo : qb0 + CW]
                                        .unsqueeze(1)
                                        .broadcast_to((64, 2, CW - qlo)),
                                        start=True,
                                        stop=True,
                                        perf_mode=DR,
                                    )
                                sched["pe"] += (2 * CW - qlo0 - qlo1) * 0.5 * CY
                                ex = expool.tile([P, 2, CW], F8, tag="ex")
                                LBL[0] = f"exp.c{c}"
                                # one exp covers the pair; for a diag pair the
                                # j1 slice [qlo0, qlo1) reads stale PSUM --
                                # bounded garbage, zeroed by the select below
                                nc.scalar.activation(
                                    ex[:, :, qlo0:CW],
                                    pss[:, :, qlo0:CW],
                                    mybir.ActivationFunctionType.Exp,
                                    scale=float(EXP_SCALE),
                                )
                                edone = max(sched["act"], sched["pe"]) + (
                                    2 * (CW - qlo0) + 222
                                ) * 0.833 + EXPC
                                sched["act"] = edone
                                pend.append(edone)
                                LBL[0] = "sel"
                                if d1:
                                    # one select covers the stale-garbage
                                    # block [qlo0,qlo1) (i-p-128<0 there) and
                                    # the diagonal triangle of j1
                                    w = qlo1 + P - qlo0
                                    nc.gpsimd.affine_select(
                                        out=ex[:, 1, qlo0 : qlo1 + P],
                                        in_=ex[:, 1, qlo0 : qlo1 + P],
                                        compare_op=mybir.AluOpType.is_ge,
                                        fill=0.0,
                                        base=qlo0 - qlo1,
                                        pattern=[[1, w]],
                                        channel_multiplier=-1,
                                    )
                                if d0:
                                    nc.gpsimd.affine_select(
                                        out=ex[:, 0, qlo0 : qlo0 + P],
                                        in_=ex[:, 0, qlo0 : qlo0 + P],
                                        compare_op=mybir.AluOpType.is_ge,
                                        fill=0.0,
                                        base=0,
                                        pattern=[[1, P]],
                                        channel_multiplier=-1,
                                    )
                                eready = edone + (900 if (d0 or d1) else 0)
                                pipe.append((eready, (jp, qlo0, ex), jp))
                                if len(pipe) > PIPED:
                                    eready, pair, jp_ = pipe.pop(0)
                                    need(eready)
                                    emit_pv(pair, jp_ == 0, False)
                                    sched["pe"] += (CW - pair[1]) * CY
                            np_ = njt // 2
                            for eready, pair, jp_ in pipe:
                                need(eready)
                                emit_pv(pair, jp_ == 0, jp_ == np_ - 1)
                                sched["pe"] += (CW - pair[1]) * CY

                            # fused normalize + eviction: 1/l -> DMA bcast ->
                            # attnT = pso * bc  (attnT holds 64*attnout)
                            LBL[0] = "norm"
                            with nc.allow_low_precision(
                                reason="1/l in bf16; enough for bf16 attnT"
                            ):
                                nc.vector.reciprocal(
                                    ltile[
                                        (h % 2) * 64 : (h % 2) * 64 + 1, h // 2, qsl
                                    ],
                                    pso[HD : HD + 1, :],
                                )
                            bc = npool.tile([P, CW], BF16, tag="bc")
                            nc.sync.dma_start(
                                out=bc[pb : pb + HD, :],
                                in_=ltile[
                                    (h % 2) * 64 : (h % 2) * 64 + 1, h // 2, qsl
                                ]
                                .unsqueeze(1)
                                .broadcast_to((1, HD, CW)),
                            )
                            nc.vector.tensor_mul(
                                attnT[pb : pb + HD, dtb, qsl],
                                pso[0:HD, :],
                                bc[pb : pb + HD, :],
                            )

                _yt_live = {}

                def wo_qt(c, qt, nt, tail=False):
                    LBL[0] = "wo"
                    qlo = c * CW + qt * P
                    if tail and (qt + nt) % 2:
                        psyt = apsum.tile([P, 2, CW], F32, tag="pss")
                        psy = psyt[:, 0, :]
                    else:
                        psyt = ps1.tile([P, CW], F32, tag="ps")
                        psy = psyt[:]
                    for dt in range(NDT):
                        nc.tensor.matmul(
                            psy,
                            attnT[:, dt, qlo : qlo + P],
                            wo_sb[:, dt, nt * CW : (nt + 1) * CW],
                            start=(dt == 0),
                            stop=(dt == NDT - 1),
                        )
                    if nt == 0:
                        yt = ypool.tile([P, 2 * CW], BF16, tag="yt")
                        _yt_live[c, qt] = yt
                    else:
                        yt = _yt_live.pop((c, qt))
                    ysl = yt[:, nt * CW : (nt + 1) * CW]
                    # in the tail phase ScalarE is idle (no more exps); use it
                    # for half the evictions so DVE doesn't serialize the
                    # drain
                    if tail and nt == 1:
                        nc.scalar.mul(ysl, psy, EVY)
                    else:
                        nc.vector.tensor_scalar_mul(ysl, psy, EVY)
                    if nt == 1:
                        nc.sync.dma_start(
                            out=y_d.ap()[qlo : qlo + P, :], in_=yt[:]
                        )

                def proj_units(xc, x8, c):
                    """Tagged thunks projecting chunk c, ordered so the FIFO
                    naturally completes half-0 K/Q and V first: [K-h0, Q-h0,
                    V, K-h1, Q-h1]."""
                    csl = slice(c * CW, (c + 1) * CW)
                    tas = {}
                    for wth, wtl, dstb, tn in (
                        (wkth, wktl, KTb, "k"),
                        (wqth, wqtl, QTb, "q"),
                    ):
                        ta = prj.tile([P, NDT, CW], BF16, tag="ta", name=f"ta_{c}_{tn}")
                        ta8 = prj8.tile([P, NDT, CW], F8, tag="ta8", name=f"ta8_{c}_{tn}")
                        tas[tn] = (ta, ta8, wth, wtl, dstb)
                    units = []
                    for half in range(2):
                        for tn in ("k", "q"):
                            ta, ta8, wth, wtl, dstb = tas[tn]
                            tag = ("kq", c, half)
                            for dt in (half, half + 2):
                                for kh, kc in ((0, 853), (1, 427)):
                                    units.append(
                                        (kc, lambda wth=wth, wtl=wtl, x8=x8,
                                         ta=ta, dt=dt, kh=kh:
                                         proj_dt_half(wth, wtl, x8, ta,
                                                      dt, kh),
                                         tag)
                                    )
                            units.append(
                                (0, lambda ta=ta, ta8=ta8, csl=csl, half=half:
                                 rope_half(ta, ta8, csl, half), tag)
                            )
                            units.append(
                                (0, lambda ta8=ta8, dstb=dstb, csl=csl,
                                 half=half: shuffle(
                                     ta8, dstb, csl,
                                     dtbs=(half * 2, half * 2 + 1)), tag)
                            )
                        if half == 0:
                            for st in range(4):
                                for vh in range(2):
                                    units.append(
                                        (853, lambda xc=xc, c=c, st=st, vh=vh:
                                         proj_v_half(xc, c, st, vh), ("v", c))
                                    )
                    return units

                def wo_units(c, tail=False):
                    units = []
                    for qt in range(4):
                        for nt in range(2):
                            units.append(
                                (1030, lambda c=c, qt=qt, nt=nt: wo_qt(
                                    c, qt, nt, tail=tail
                                ), ("wo", c))
                            )
                    return units

                def ensure_for(c, halves):
                    def pred(tag):
                        if tag[0] == "kq":
                            return tag[1] < c or (tag[1] == c and
                                                  tag[2] in halves)
                        if tag[0] == "v":
                            return tag[1] <= c
                        return False
                    ensure(pred)

                # Linearized schedule: chunk-0 K-h0/Q-h0/V projected
                # serially, then attention parts stream while the global FIFO
                # drip-feeds proj/wo work into PE stalls.  ensure_for()
                # prefix-drains the FIFO before each part that consumes it.
                xcs = {0: xc0}
                x8s = {0: x80}
                xc1 = xw.tile([P, NKT, CW], BF16, tag="xc")
                x81 = xw8.tile([P, NKT, 2, CW], F8, tag="x8c")
                xcs[1] = xc1
                x8s[1] = x81
                pu = proj_units(xc0, x80, 0)
                for ucost, u, _t in pu[0:12]:
                    u()
                    sched["pe"] += ucost
                pending.extend(pu[12:])
                attention(0, dtbs=range(0, 2))
                nc.sync.dma_start(out=cost[:, CW:S], in_=cos_d.ap()[:, CW:S])
                nc.scalar.dma_start(out=sint[:, CW:S], in_=sin_d.ap()[:, CW:S])
                nc.sync.dma_start(out=x81[:], in_=x8_v[1])
                nc.sync.dma_start(out=xc1[:], in_=xT_v[:, :, CW : 2 * CW])
                nc.scalar.dma_start(out=wo_sb[:], in_=wo_v[:])

                pending.extend(proj_units(xc1, x81, 1))
                xc2 = xw.tile([P, NKT, CW], BF16, tag="xc")
                x82 = xw8.tile([P, NKT, 2, CW], F8, tag="x8c")
                xcs[2] = xc2
                x8s[2] = x82
                pending.append(
                    (0, lambda: (nc.sync.dma_start(out=x82[:], in_=x8_v[2]),
                                 nc.sync.dma_start(
                        out=xc2[:], in_=xT_v[:, :, 2 * CW : 3 * CW])),
                     ("xc", 2)))
                ensure_for(0, {1})
                attention(0, dtbs=range(2, 4))
                ensure_for(1, {0})
                attention(1, dtbs=range(0, 2))

                pending.extend(proj_units(xc2, x82, 2))
                xc3 = xw.tile([P, NKT, CW], BF16, tag="xc")
                x83 = xw8.tile([P, NKT, 2, CW], F8, tag="x8c")
                xcs[3] = xc3
                x8s[3] = x83
                pending.append(
                    (0, lambda: (nc.sync.dma_start(out=x83[:], in_=x8_v[3]),
                                 nc.sync.dma_start(
                        out=xc3[:], in_=xT_v[:, :, 3 * CW : 4 * CW])),
                     ("xc", 3)))
                ensure_for(1, {1})
                attention(1, dtbs=range(2, 4))
                ensure_for(2, {0})
                attention(2, dtbs=range(0, 2))

                pending.extend(proj_units(xc3, x83, 3))
                ensure_for(2, {1})
                attention(2, dtbs=range(2, 4))
                pending.extend(wo_units(0) + wo_units(1) + wo_units(2))
                ensure_for(3, {0})
                attention(3, dtbs=range(0, 1))
                ensure_for(3, {0, 1})
                attention(3, dtbs=range(1, 4))
                while pending:
                    run_unit()
                for ucost, u, _t in wo_units(NSC - 1, tail=True):
                    u()
                if os.environ.get("KV3_SCHED") == "1":
                    print(f"[cursor model] pe={sched['pe']:.0f} act={sched['act']:.0f}")
                if DBG:
                    nc.sync.dma_start(out=dbg_ktb.ap(), in_=KTb[:])
                    nc.sync.dma_start(out=dbg_qtb.ap(), in_=QTb[:])
                    nc.sync.dma_start(out=dbg_vh.ap(), in_=V8h[:])
                    nc.sync.dma_start(out=dbg_vl.ap(), in_=V8l[:])
                    nc.sync.dma_start(out=dbg_attnT.ap(), in_=attnT[:])
                    nc.sync.dma_start(out=dbg_ltile.ap(), in_=ltile[:])

    nc.compile()
    return nc


def _perm_a():
    """Column permutation for wq/wk: even head-dims of all heads first
    (head-major, 32 per head), then odd head-dims."""
    perm = np.empty(DG, dtype=np.int64)
    for n in range(DG):
        if n < DG // 2:
            h, i = n // 32, n % 32
            perm[n] = h * HD + 2 * i
        else:
            h, i = (n - DG // 2) // 32, (n - DG // 2) % 32
            perm[n] = h * HD + 2 * i + 1
    return perm


def _pack_w(w):
    """[D, DG] -> [NDT, P(contraction row), NKT, P(col)] so per-dt loads have
    2KB-contiguous runs on both DMA sides."""
    w4 = w.reshape(NKT, P, NDT, P)           # (kt, p, dt, m)
    return np.ascontiguousarray(w4.transpose(2, 1, 0, 3))


def _split8(a, s):
    """scaled e4m3 hi/lo split of fp32 array (fp8 pair, descale-free)."""
    f8 = ml_dtypes.float8_e4m3
    hi = (a * s).astype(f8)
    lo = ((a * s).astype(np.float32) - hi.astype(np.float32)).astype(f8)
    return hi, lo


def kernel(**inputs):
    global _PROGRAM
    x = np.asarray(inputs["x"], dtype=np.float32)
    freqs_cos = np.asarray(inputs["freqs_cos"], dtype=np.float32)
    freqs_sin = np.asarray(inputs["freqs_sin"], dtype=np.float32)
    wq = np.asarray(inputs["wq"], dtype=np.float32)
    wk = np.asarray(inputs["wk"], dtype=np.float32)
    wv = np.asarray(inputs["wv"], dtype=np.float32)
    wo = np.asarray(inputs["wo"], dtype=np.float32)

    if _PROGRAM is None:
        _PROGRAM = _build_program()
    nc = _PROGRAM

    bf = ml_dtypes.bfloat16
    perm = _perm_a()
    cost = np.ascontiguousarray(np.tile(freqs_cos.T, (4, 1))).astype(bf)
    sint = np.ascontiguousarray(np.tile(freqs_sin.T, (4, 1))).astype(bf)

    SX, SW = 32.0, 2048.0
    xsp = {}
    for b in range(B):
        hi, lo = _split8(np.ascontiguousarray(x[b].T), SX)
        a = np.stack([hi, lo], axis=1)                    # [D, 2, S]
        a = a.reshape(NKT, P, 2, NSC, CW).transpose(3, 0, 1, 2, 4)
        xsp[b] = np.ascontiguousarray(a)                  # [NSC, NKT, P, 2, CW]
    wsp = {}
    for g in range(2):
        gsl = slice(g * DG, (g + 1) * DG)
        qh, ql = _split8(wq[:, gsl][:, perm], SW)
        kh, kl = _split8(wk[:, gsl][:, perm], SW)
        wsp[g] = {
            "wq8h": _pack_w(qh), "wq8l": _pack_w(ql),
            "wk8h": _pack_w(kh), "wk8l": _pack_w(kl),
            "wv": np.ascontiguousarray(wv[:, gsl]).astype(bf),
            "wo": np.ascontiguousarray(wo[gsl, :]).astype(bf),
        }
    in_maps = []
    for c in range(NCORES):
        b, g = c // 2, c % 2
        in_maps.append(
            {
                "xT": np.ascontiguousarray(x[b].T).astype(bf),
                "x8": xsp[b],
                "cost": cost,
                "sint": sint,
                **wsp[g],
            }
        )

    res = run_bass_kernel_spmd(nc, in_maps, list(range(NCORES)))
    global _LAST_RESULTS
    _LAST_RESULTS = res.results
    y = np.empty((B, S, D), dtype=np.float32)
    for b in range(B):
        y[b] = res.results[2 * b]["y"].astype(np.float32) + res.results[
            2 * b + 1
        ]["y"].astype(np.float32)
    return y
